# revision 1
# baseline (speedup 1.0000x reference)
"""Trainium2 Bass kernel for ContextualAttentionModule.

Data-parallel over batch: 8 samples -> 8 NeuronCores, one sample per core.
Per-core pipeline (C=256, H=W=32, L=1024 patches):
  scores  = <fg_patch(p), bg_patch(l)>  via fp8e4 DoubleRow matmuls
            (K=256 channel pairs per pass), hi/lo 3-term compensated
            (bh*fh + bh*fl + bl*fh); x-shift variant tiles give the
            flat weight APs DoubleRow requires
  norm    = sqrt(3x3-boxsum(colsum bg_masked^2) + 2304*eps^2); the
            [128,8] per-partition layout comes from a PE broadcast and
            identity-masked diagonal extraction
  prop    = 3x3 window-sum of scores + eps*boxbox(colsum fg)  (DVE, bf16)
  E       = exp(prop * rn)  in bf16 (no max-subtract)
  D, s2   = interleaved per-block [1,512] PE accumulations over E
  attn    = E * (1/D) broadcast (PE ones-outer-product), in place
  recov   = tconv(attn, kernels/norm) f32r... bf16, via PE-transposed
            shifted patch banks
  final   = (recov + eps*box(s2/D))*mask/9 + fg*(1-mask)
  out     = concat_g relu(dilated_conv_r(final) + b)  bf16 weights

fg/bg are uploaded in bf16 (halves load DMA); first three score blocks
run hi*hi-only with lo-term correction groups two iterations later so
the PE starts before the lo quantization chain lands.
"""

import numpy as np
import ml_dtypes

import concourse.bass as bass
import concourse.tile as tile
from concourse import bacc, mybir
from concourse.bass_utils import run_bass_kernel_spmd
from concourse.masks import make_identity

F32 = mybir.dt.float32
F32R = mybir.dt.float32r
BF16 = mybir.dt.bfloat16
F8 = mybir.dt.float8e4
U8 = mybir.dt.uint8
U16 = mybir.dt.uint16
AF = mybir.ActivationFunctionType
ALU = mybir.AluOpType
DR = mybir.MatmulPerfMode.DoubleRow

EPS = 1e-7
RATES = (1, 2, 4, 8)
OFFS = [(dy, dx) for dy in range(3) for dx in range(3)]
# dy=1 taps first so the start=True matmul covers the full psum tile
TAPS = [(1, 0), (1, 1), (1, 2), (0, 0), (0, 1), (0, 2), (2, 0), (2, 1), (2, 2)]

_CACHE = {}


def _ring_zero(nc, buf, n=34, eng=None):
    """Zero only the 1-wide border ring of a [P, n, n] padded buffer."""
    eng = eng or nc.vector
    bc = U16 if buf.dtype == BF16 else F32
    eng.memset(buf[:, 0:n:n - 1, :].bitcast(bc), 0)
    eng.memset(buf[:, 1:n - 1, 0:n:n - 1].bitcast(bc), 0)


def _boxsum(nc, scr, src_pad, dst_flat, eng=None):
    """3x3 SAME window sum: [1,34,34] ring-zero padded -> [1,32,32] flat."""
    eng = eng or nc.vector
    eng.tensor_tensor(scr[:, 1:33, 1:33], src_pad[:, 1:33, 0:32],
                      src_pad[:, 1:33, 1:33], ALU.add)
    eng.tensor_tensor(scr[:, 1:33, 1:33], scr[:, 1:33, 1:33],
                      src_pad[:, 1:33, 2:34], ALU.add)
    eng.tensor_tensor(dst_flat[:], scr[:, 0:32, 1:33],
                      scr[:, 1:33, 1:33], ALU.add)
    eng.tensor_tensor(dst_flat[:], dst_flat[:], scr[:, 2:34, 1:33], ALU.add)


def build_program(debug=False):
    nc = bacc.Bacc()
    fg_d = nc.declare_dram_parameter("fg", [2, 128, 32, 32], BF16, isOutput=False)
    bg_d = nc.declare_dram_parameter("bg", [2, 128, 32, 32], BF16, isOutput=False)
    mask_d = nc.declare_dram_parameter("maskrow", [1, 1024], F32, isOutput=False)
    w_d = nc.declare_dram_parameter("wconv", [2, 128, 2304], BF16, isOutput=False)
    b_d = nc.declare_dram_parameter("bias", [256, 1], F32, isOutput=False)
    out_d = nc.declare_dram_parameter("out", [256, 32, 32], F32, isOutput=True)
    dbg = {}
    if debug:
        for nm, shp in [("d_scores", [128, 32, 32]), ("d_rn", [128, 8]),
                        ("d_E", [128, 32, 32]), ("d_drow", [1, 1024]),
                        ("d_prec", [128, 512]), ("d_final", [128, 32, 32]),
                        ("d_gb", [128, 32, 32])]:
            dbg[nm] = nc.declare_dram_parameter(nm, shp, F32, isOutput=True)

    with tile.TileContext(nc) as tc:
        _emit(nc, tc, fg_d, bg_d, mask_d, w_d, b_d, out_d, dbg)
    nc.compile()
    return nc


def _emit(nc, tc, fg_d, bg_d, mask_d, w_d, b_d, out_d, dbg=None):
    dbg = dbg or {}
    with tc.tile_pool(name="main", bufs=1) as main:
        # ---------------- long-lived tiles ----------------
        fflat = main.tile([128, 2, 32, 32], BF16, name="fflat")
        invmaskb = main.tile([128, 32, 32], F32, name="invmaskb")
        maskb9 = main.tile([128, 32, 32], F32, name="maskb9")
        ones_col = main.tile([128, 1], F32R, name="ones_col")
        idR = main.tile([128, 128], BF16, name="idR")
        rncol = main.tile([128, 8], F32, name="rncol")
        ercol = main.tile([128, 8], BF16, name="ercol")
        onesB = main.tile([128, 1], BF16, name="onesB")
        Gb = main.tile([128, 32, 32], BF16, name="Gb")
        msrow = main.tile([1, 1024], F32, name="msrow")
        A = [main.tile([128, 34, 34], BF16, name=f"A{t}") for t in range(8)]
        W = [main.tile([128, 34, 34], BF16, name=f"W{i}") for i in range(2)]

        with tc.tile_pool(name="work", bufs=1) as work:
            bgs = [[work.tile([128, 32, 32], BF16, name=f"bgs{c}_{d}")
                    for d in range(9)] for c in range(2)]
            bgTp_cm = tc.tile_pool(name="bgTp", bufs=5)
            bgTp = bgTp_cm.__enter__()
            scorep_cm = tc.tile_pool(name="scorep", bufs=1)
            scorep = scorep_cm.__enter__()
            ps_sc_cm = tc.tile_pool(name="ps_sc", bufs=2, space="PSUM")
            ps_sc = ps_sc_cm.__enter__()
            ps_acc_cm = tc.tile_pool(name="ps_acc", bufs=1, space="PSUM")
            ps_acc = ps_acc_cm.__enter__()
            ps_tra_cm = tc.tile_pool(name="ps_tra", bufs=2, space="PSUM")
            ps_tra = ps_tra_cm.__enter__()
            # x-shift variants: tile v holds value x[w + v - 1] (0 at edges);
            # y handled by row offsets into the 34-row padded (bg) / 32-row
            # (fg, row-trimmed) layouts.
            bg8h = [scorep.tile([128, 2, 34, 32], F8, name=f"bg8h{v}")
                    for v in range(3)]
            bg8l = [scorep.tile([128, 2, 34, 32], F8, name=f"bg8l{v}")
                    for v in range(3)]
            fg8h = [scorep.tile([128, 2, 32, 32], F8, name=f"fg8h{v}")
                    for v in range(3)]
            fg8l = [scorep.tile([128, 2, 32, 32], F8, name=f"fg8l{v}")
                    for v in range(3)]

            stage_cm = tc.tile_pool(name="stage", bufs=1)
            stage = stage_cm.__enter__()
            if True:
                bq = stage.tile([128, 2, 32, 32], BF16, name="bq")
                bgsq = stage.tile([128, 2, 32, 32], F32R, name="bgsq")
                # ---------- phase 0: loads first, then prioritized quant ----------
                onesf0 = stage.tile([128, 1], F32, name="onesf0")
                nc.sync.dma_start(msrow[:], mask_d[:])
                nc.gpsimd.dma_start(bq[:, 0], bg_d[0])
                nc.sync.dma_start(bq[:, 1], bg_d[1])
                nc.scalar.dma_start(fflat[:, 0], fg_d[0])
                nc.sync.dma_start(fflat[:, 1], fg_d[1])
                nc.gpsimd.memset(onesf0[:], 1.0)
                # preload the Exp activation table off the critical path
                nc.scalar.activation(onesf0[0:1], onesf0[0:1], AF.Exp)
                nc.gpsimd.memset(onesf0[:], 1.0)
                # mask broadcast via PE ones-outer-product (PE idle here; the
                # gpsimd Q7 broadcast is slow to become ready)
                onesrowR = stage.tile([1, 128], F32R, name="onesrowR")
                nc.vector.memset(onesrowR[:].bitcast(F32), 1.0)
                msrowR = stage.tile([1, 1024], F32R, name="msrowR")
                nc.vector.tensor_copy(msrowR[:], msrow[:])
                invmaskB = stage.tile([128, 32, 32], BF16, name="invmaskB")
                for h in range(2):
                    psb = ps_sc.tile([128, 16, 32], F32, name="psb", tag="psc")
                    nc.tensor.matmul(
                        psb[:].rearrange("p a b -> p (a b)"), onesrowR[:],
                        msrowR[:, 512 * h:512 * (h + 1)], start=True, stop=True)
                    nc.vector.tensor_scalar(
                        out=invmaskB[:, 16 * h:16 * (h + 1), :], in0=psb[:],
                        scalar1=-1.0, scalar2=1.0, op0=ALU.mult, op1=ALU.add)

                # fg hi: center + one variant on Act, one on DVE
                nc.scalar.copy(fg8h[1][:], fflat[:])
                nc.scalar.copy(fg8h[2][:, :, :, 0:31], fflat[:, :, :, 1:32])
                nc.vector.tensor_copy(fg8h[0][:, :, :, 1:32],
                                      fflat[:, :, :, 0:31])

                # bg hi center: fused mask+quantize from the RAW loads (DVE)
                for j in range(2):
                    nc.vector.scalar_tensor_tensor(
                        out=bg8h[1][:, j, 1:33, :], in0=bq[:, j], scalar=1.0,
                        in1=invmaskB[:], op0=ALU.mult, op1=ALU.mult)
                # bg hi variants: byte copies of the center (Pool + Act)
                nc.gpsimd.tensor_copy(
                    bg8h[2][:, :, 1:33, 0:31].bitcast(U8),
                    bg8h[1][:, :, 1:33, 1:32].bitcast(U8))
                nc.scalar.copy(bg8h[0][:, :, 1:33, 1:32],
                               bg8h[1][:, :, 1:33, 0:31])

                # mask bq in place (feeds lo residuals, bg^2, bgs)
                nc.vector.tensor_tensor(bq[:, 0], bq[:, 0], invmaskB[:],
                                        ALU.mult)
                nc.vector.tensor_tensor(bq[:, 1], bq[:, 1], invmaskB[:],
                                        ALU.mult)

                # lo residuals: lo = x - f32(hi), fused STT reading fp8
                nc.vector.scalar_tensor_tensor(
                    out=fg8l[1][:], in0=fg8h[1][:], scalar=-1.0, in1=fflat[:],
                    op0=ALU.mult, op1=ALU.add)
                nc.vector.scalar_tensor_tensor(
                    out=bg8l[1][:, :, 1:33, :], in0=bg8h[1][:, :, 1:33, :],
                    scalar=-1.0, in1=bq[:], op0=ALU.mult, op1=ALU.add)
                nc.scalar.copy(fg8l[2][:, :, :, 0:31], fg8l[1][:, :, :, 1:32])
                nc.vector.tensor_copy(fg8l[0][:, :, :, 1:32],
                                      fg8l[1][:, :, :, 0:31])
                nc.scalar.copy(bg8l[2][:, :, 1:33, 0:31],
                               bg8l[1][:, :, 1:33, 1:32])
                nc.gpsimd.tensor_copy(
                    bg8l[0][:, :, 1:33, 1:32].bitcast(U8),
                    bg8l[1][:, :, 1:33, 0:31].bitcast(U8))

                # edge/pad memsets split across Act/DVE idle windows
                for i, t8 in enumerate(bg8h + bg8l):
                    eng = nc.vector if i % 2 else nc.scalar
                    if eng is nc.scalar:
                        nc.scalar.memzero(t8[:, :, 0:34:33, :])
                    else:
                        eng.memset(t8[:, :, 0:34:33, :].bitcast(U8), 0)
                for lst in (bg8h, bg8l):
                    nc.vector.memset(lst[2][:, :, 1:33, 31:32].bitcast(U8), 0)
                    nc.vector.memset(lst[0][:, :, 1:33, 0:1].bitcast(U8), 0)
                for lst in (fg8h, fg8l):
                    nc.vector.memset(lst[2][:, :, :, 31:32].bitcast(U8), 0)
                    nc.vector.memset(lst[0][:, :, :, 0:1].bitcast(U8), 0)
                for t in range(8):
                    _ring_zero(nc, A[t],
                               eng=(nc.vector if t % 2 else nc.gpsimd))
                for w in W:
                    _ring_zero(nc, w, eng=nc.gpsimd)
                nc.vector.tensor_copy(ones_col[:], onesf0[:])
                nc.gpsimd.tensor_copy(onesB[:], onesf0[:])

                # bg^2 (for patch norms)
                nc.scalar.square(bgsq[:], bq[:])
                idf = stage.tile([128, 128], F32, name="idf")
                make_identity(nc, idf[:])
                nc.gpsimd.tensor_copy(idR[:], idf[:])

                # bf16 shifted masked-bg tiles (transpose sources; first
                # consumer is build_bgT(0) emitted at scores iteration 3)
                k = 0
                for cb in range(2):
                    for d, (dy, dx) in enumerate(OFFS):
                        f = bgs[cb][d]
                        r0, r1 = max(0, 1 - dy), min(32, 33 - dy)
                        c0, c1 = max(0, 1 - dx), min(32, 33 - dx)
                        eng = (None, nc.vector, nc.vector, nc.gpsimd)[k % 4]
                        if r0 == 1:
                            nc.gpsimd.memset(f[:, 0:1, :].bitcast(U16), 0)
                        if r1 == 31:
                            nc.gpsimd.memset(f[:, 31:32, :].bitcast(U16), 0)
                        if c0 == 1:
                            nc.gpsimd.memset(f[:, :, 0:1].bitcast(U16), 0)
                        if c1 == 31:
                            nc.gpsimd.memset(f[:, :, 31:32].bitcast(U16), 0)
                        src = bq[:, cb, r0 + dy - 1:r1 + dy - 1,
                                 c0 + dx - 1:c1 + dx - 1]
                        if eng is None:
                            nc.scalar.copy(f[:, r0:r1, c0:c1], src)
                        else:
                            eng.tensor_copy(f[:, r0:r1, c0:c1], src)
                        k += 1

                def emit_phase1(PS1):
                    rpad = stage.tile([1, 34, 34], F32R, name="rpad")
                    rscr = stage.tile([1, 34, 34], F32R, name="rscr")
                    _ring_zero(nc, rpad, eng=nc.gpsimd)
                    _ring_zero(nc, rscr, eng=nc.gpsimd)
                    # ssq row = ones^T @ bg^2, then 3x3 boxsum -> patch norms
                    for ch in range(2):
                        pr = PS1.tile([1, 512], F32, name="pr", tag="psc")
                        for cb in range(2):
                            nc.tensor.matmul(
                                pr[:], ones_col[:],
                                bgsq[:, cb, 16 * ch:16 * ch + 16, :],
                                start=(cb == 0), stop=(cb == 1))
                        nc.scalar.copy(
                            rpad[:, 1 + 16 * ch:17 + 16 * ch, 1:33], pr[:])
                    ssqrow = stage.tile([1, 32, 32], F32R, name="ssqrow")
                    _boxsum(nc, rscr, rpad, ssqrow)
                    urow = stage.tile([1, 1024], F32, name="urow")
                    nc.vector.tensor_scalar_add(
                        urow[:], ssqrow[:].rearrange("o a b -> o (a b)"),
                        2304.0 * EPS * EPS)
                    nc.scalar.activation(urow[:], urow[:], AF.Sqrt)
                    nc.vector.reciprocal(urow[:], urow[:])
                    # rncol[p, t] = rnrow[128t + p]: broadcast + diag extract
                    Bb = stage.tile([128, 1024], F32, name="Bb")
                    nc.gpsimd.partition_broadcast(Bb[:], urow[:])
                    junk = stage.tile([128, 128], F32, name="junk")
                    for t in range(8):
                        nc.vector.scalar_tensor_tensor(
                            out=junk[:], in0=Bb[:, 128 * t:128 * (t + 1)],
                            scalar=1.0, in1=idf[:], op0=ALU.mult, op1=ALU.mult,
                            accum_out=rncol[:, t:t + 1])
                    nc.vector.tensor_scalar_mul(ercol[:], rncol[:], EPS)

                    # boxg: G = 3x3 box of channel-sum(fg); Gb = eps*box(G) bcast
                    ones8 = stage.tile([128, 2, 16], F8, name="ones8")
                    nc.gpsimd.memset(ones8[:].bitcast(U8), 0)
                    nc.gpsimd.memset(ones8[:, :, 0:1], 1.0)
                    for ch in range(2):
                        prg = PS1.tile([16, 512], F32, name="prg", tag="psc")
                        for li, lvl in enumerate((fg8h, fg8l)):
                            nc.tensor.matmul(
                                prg[:], ones8[:],
                                lvl[1][:, :, 16 * ch:16 * ch + 16, :],
                                start=(li == 0), stop=(li == 1), perf_mode=DR)
                        nc.scalar.copy(
                            rpad[:, 1 + 16 * ch:17 + 16 * ch, 1:33], prg[0:1])
                    _boxsum(nc, rscr, rpad, ssqrow)
                    nc.vector.tensor_copy(rpad[:, 1:33, 1:33], ssqrow[:])
                    bbrow = stage.tile([1, 32, 32], F32R, name="bbrow")
                    _boxsum(nc, rscr, rpad, bbrow)
                    ebrowB = stage.tile([1, 1024], BF16, name="ebrowB")
                    nc.vector.tensor_scalar_mul(
                        ebrowB[:], bbrow[:].rearrange("o a b -> o (a b)"), EPS)
                    nc.gpsimd.partition_broadcast(
                        Gb.rearrange("p a b -> p (a b)"), ebrowB[:])

                    if dbg:
                        nc.gpsimd.dma_start(dbg["d_rn"][:], rncol[:])
                        nc.gpsimd.dma_start(dbg["d_gb"][:], Gb[:].bitcast(F32))


            # ---------- scores loop (fp8 DR) with interleaved D/s2 ----------
            bgT = {}
            blocks = [(cb, d) for cb in range(2) for d in range(9)]

            if True:
                hp_cm = tc.tile_pool(name="hp", bufs=2)
                hp = hp_cm.__enter__()

                def build_bgT(t, pool):
                    bgT[t] = bgTp.tile([128, 2304], BF16, name="bgT", tag="bgT")
                    for grp in range(5):
                        chunk = blocks[4 * grp:4 * grp + 4]
                        ptr = pool.tile([128, 512], BF16, name="ptr", tag="ptr")
                        for bi, (cb, d) in enumerate(chunk):
                            nc.tensor.transpose(
                                ptr[:, 128 * bi:128 * (bi + 1)],
                                bgs[cb][d].rearrange("p a b -> p (a b)")
                                [:, 128 * t:128 * (t + 1)],
                                idR[:])
                        n = len(chunk)
                        nc.scalar.activation(
                            bgT[t][:, 512 * grp:512 * grp + 128 * n],
                            ptr[:, :128 * n], AF.Copy, scale=rncol[:, t:t + 1])

                def emit_boxsum_exp(t):
                    w = W[t % 2]
                    nc.vector.tensor_tensor(
                        w[:, 1:33, 1:33], A[t][:, 1:33, 0:32],
                        A[t][:, 1:33, 1:33], ALU.add)
                    nc.vector.tensor_tensor(
                        w[:, 1:33, 1:33], w[:, 1:33, 1:33],
                        A[t][:, 1:33, 2:34], ALU.add)
                    H = hp.tile([128, 32, 32], BF16, name="H", tag="H")
                    nc.vector.tensor_tensor(
                        H[:], w[:, 0:32, 1:33], w[:, 1:33, 1:33], ALU.add)
                    nc.vector.tensor_tensor(H[:], H[:], w[:, 2:34, 1:33], ALU.add)
                    nc.vector.tensor_tensor(H[:], H[:], Gb[:], ALU.add)
                    nc.scalar.activation(A[t][:, 1:33, 1:33], H[:], AF.Exp,
                                         scale=rncol[:, t:t + 1])

                rdrow = work.tile([1, 1024], F32, name="rdrow")
                s2row = work.tile([1, 1024], F32, name="s2row")
                if True:
                    dAcc = [ps_acc.tile([1, 512], F32, name=f"dAcc{c}")
                            for c in range(2)]
                    s2Acc = [ps_acc.tile([1, 512], F32, name=f"s2Acc{c}")
                             for c in range(2)]

                    def emit_scores_terms(t, ch, terms, accum):
                        psc = ps_sc.tile([128, 16, 32], F32, name="psc",
                                         tag="psc")
                        n = 9 * len(terms)
                        i = 0
                        for lhs8, rhs8 in terms:
                            for dy, dx in TAPS:
                                fr = 16 * ch + dy - 1
                                r0, r1 = max(0, -fr), min(16, 32 - fr)
                                nc.tensor.matmul(
                                    psc[:, r0:r1, :],
                                    lhs8[dx][:, :, 4 * t + dy:4 * t + dy + 4, :],
                                    rhs8[dx][:, :, fr + r0:fr + r1, :],
                                    start=(i == 0), stop=(i == n - 1),
                                    perf_mode=DR)
                                i += 1
                        half = A[t][:, 1 + 16 * ch:17 + 16 * ch, 1:33]
                        if accum:
                            nc.vector.tensor_tensor(half, half, psc[:], ALU.add)
                        elif ch == 0:
                            nc.scalar.copy(half, psc[:])
                        else:
                            nc.vector.tensor_copy(half, psc[:])

                    T_HH = ((bg8h, fg8h),)
                    T_LO = ((bg8h, fg8l), (bg8l, fg8h))
                    T_ALL = T_HH + T_LO

                    def emit_scores(t, ch):
                        emit_scores_terms(t, ch, T_ALL, False)

                    def emit_d_s2(t):
                        for ch in range(2):
                            av = A[t][:, 1 + 16 * ch:17 + 16 * ch, 1:33]
                            nc.tensor.matmul(dAcc[ch][:], onesB[:], av,
                                             start=(t == 0), stop=(t == 7))
                            nc.tensor.matmul(s2Acc[ch][:], ercol[:, t:t + 1],
                                             av, start=(t == 0), stop=(t == 7))

                    for t in range(8):
                        emit_scores(t, 0)
                        if t >= 1:
                            emit_d_s2(t - 1)
                        emit_scores(t, 1)
                        if t == 0:
                            emit_phase1(ps_sc)
                        if t >= 3:
                            build_bgT(t - 3, ps_tra)
                        emit_boxsum_exp(t)
                        if dbg and t == 0:
                            nc.gpsimd.dma_start(
                                dbg["d_E"][:], A[0][:, 1:33, 1:33])
                    emit_d_s2(7)

                    # softmax denominator + s2 rows -> SBUF
                    for ch in range(2):
                        nc.vector.reciprocal(
                            rdrow[:, 512 * ch:512 * (ch + 1)], dAcc[ch][:])
                        nc.vector.tensor_tensor(
                            s2row[:, 512 * ch:512 * (ch + 1)], s2Acc[ch][:],
                            rdrow[:, 512 * ch:512 * (ch + 1)], ALU.mult)
                    if dbg:
                        nc.gpsimd.dma_start(dbg["d_drow"][:], rdrow[:])
                # ---------- scores psum pools closed ----------
                hp_cm.__exit__(None, None, None)
                stage_cm.__exit__(None, None, None)
                ps_tra_cm.__exit__(None, None, None)
                ps_acc_cm.__exit__(None, None, None)
                ps_sc_cm.__exit__(None, None, None)
                scorep_cm.__exit__(None, None, None)

                with tc.tile_pool(name="tail", bufs=1) as tail:
                    nc.gpsimd.tensor_copy(invmaskb[:], invmaskB[:])
                    nc.gpsimd.tensor_scalar(
                        out=maskb9[:], in0=invmaskB[:], scalar1=-1.0 / 9.0,
                        scalar2=1.0 / 9.0, op0=ALU.mult, op1=ALU.add)
                    rdrowB = tail.tile([1, 1024], BF16, name="rdrowB")
                    nc.vector.tensor_copy(rdrowB[:], rdrow[:])
                    Db = tail.tile([128, 32, 32], BF16, name="Db")
                    nc.gpsimd.partition_broadcast(
                        Db.rearrange("p a b -> p (a b)"), rdrowB[:])
                    s2pad = W[0][0:1]
                    s2scr = W[1][0:1]
                    _ring_zero(nc, s2pad, eng=nc.gpsimd)
                    _ring_zero(nc, s2scr, eng=nc.gpsimd)
                    nc.gpsimd.tensor_copy(
                        s2pad[:, 1:33, 1:33],
                        s2row[:].rearrange("o (a b) -> o a b", b=32))
                    boxs2 = tail.tile([1, 32, 32], BF16, name="boxs2")
                    _boxsum(nc, s2scr, s2pad, boxs2, eng=nc.gpsimd)
                    nc.vector.tensor_scalar_mul(
                        s2row[:], boxs2[:].rearrange("o a b -> o (a b)"), EPS)
                    s2b = tail.tile([128, 32, 32], F32, name="s2b")
                    nc.gpsimd.partition_broadcast(
                        s2b.rearrange("p a b -> p (a b)"), s2row[:])
                    nc.vector.tensor_tensor(s2b[:], s2b[:], maskb9[:], ALU.mult)

                    # ---------- attn normalize + tconv ----------
                    with (
                        tc.tile_pool(name="ps_rec", bufs=1,
                                     space="PSUM") as psrec,
                        tc.tile_pool(name="ps_trb", bufs=2,
                                     space="PSUM") as ps_trb,
                    ):
                        prec = [[psrec.tile([128, 512], F32,
                                            name=f"prec{c}_{ch}")
                                 for ch in range(2)] for c in range(2)]

                        def emit_tconv(t):
                            for cb in range(2):
                                for ch in range(2):
                                    for d, (dy, dx) in enumerate(OFFS):
                                        z0 = 16 * ch + 2 - dy
                                        x0 = 2 - dx
                                        nc.tensor.matmul(
                                            prec[cb][ch][:],
                                            bgT[t][:, 128 * (9 * cb + d):
                                                   128 * (9 * cb + d + 1)],
                                            A[t][:, z0:z0 + 16, x0:x0 + 32],
                                            start=(t == 0 and d == 0),
                                            stop=(t == 7 and d == 8))

                        for t in range(8):
                            nc.vector.tensor_tensor(
                                A[t][:, 1:33, 1:33], A[t][:, 1:33, 1:33],
                                Db[:], ALU.mult)
                        for t in range(8):
                            emit_tconv(t)
                            if t + 4 < 8:
                                build_bgT(t + 4, ps_trb)
                        bgT.clear()

                        # final = rec*mask/9 + eps*boxs2*mask/9 + fg*(1-mask)
                        final_pad = [main.tile([128, 48, 48], BF16,
                                               name=f"final_pad{c}")
                                     for c in range(2)]
                        for c in range(2):
                            nc.vector.memset(
                                final_pad[c][:, 0:8, :].bitcast(U16), 0)
                            nc.vector.memset(
                                final_pad[c][:, 40:48, :].bitcast(U16), 0)
                            nc.gpsimd.memset(
                                final_pad[c][:, 8:40, 0:8].bitcast(U16), 0)
                            nc.gpsimd.memset(
                                final_pad[c][:, 8:40, 40:48].bitcast(U16), 0)
                        fscr = [tail.tile([128, 32, 32], F32, name=f"fscr{i}")
                                for i in range(2)]
                        for cb in range(2):
                            for ch in range(2):
                                r0 = 16 * ch
                                nc.vector.tensor_tensor(
                                    fscr[0][:, r0:r0 + 16, :], prec[cb][ch][:],
                                    maskb9[:, r0:r0 + 16, :], ALU.mult)
                            nc.vector.tensor_tensor(fscr[0][:], fscr[0][:],
                                                    s2b[:], ALU.add)
                            nc.gpsimd.tensor_tensor(fscr[1][:], fflat[:, cb],
                                                    invmaskb[:], ALU.mult)
                            nc.vector.tensor_tensor(
                                final_pad[cb][:, 8:40, 8:40],
                                fscr[0][:], fscr[1][:], ALU.add)
                        if dbg:
                            nc.vector.tensor_copy(
                                fscr[1][:, 0:16, :], prec[0][0][:])
                            nc.gpsimd.dma_start(
                                dbg["d_prec"][:],
                                fscr[1].rearrange("p a b -> p (a b)")[:, 0:512])
                            nc.gpsimd.dma_start(dbg["d_final"][:],
                                                final_pad[0][:, 8:40, 8:40])
            bgTp_cm.__exit__(None, None, None)
        # ---------- work pool closed (scores tiles freed) ----------

        with tc.tile_pool(name="late", bufs=1) as late:
            # ---------- dilated convs (bf16) ----------
            wsb = [late.tile([128, 2304], BF16, name=f"wsb{c}") for c in range(2)]
            biasb = [late.tile([128, 1], F32, name=f"biasb{c}") for c in range(2)]
            for c in range(2):
                nc.scalar.dma_start(wsb[c][:], w_d[c])
                nc.scalar.dma_start(biasb[c][:], b_d[128 * c:128 * (c + 1)])
            out_sb = [late.tile([128, 32, 32], F32, name=f"out_sb{c}")
                      for c in range(2)]

            with tc.tile_pool(name="ps_o", bufs=3, space="PSUM") as pso_pool:
                for ct_out in range(2):
                    for ch in range(2):
                        pso = pso_pool.tile([128, 512], F32, name="pso", tag="pso")
                        for half in range(2):
                            g = 2 * ct_out + half
                            r = RATES[g]
                            i = 0
                            for c in range(2):
                                for d, (dy, dx) in enumerate(OFFS):
                                    oy = 8 + r * (dy - 1) + 16 * ch
                                    ox = 8 + r * (dx - 1)
                                    woff = 576 * g + 64 * (3 * dy + dx)
                                    nc.tensor.matmul(
                                        pso[64 * half:64 * half + 64, :],
                                        wsb[c][:, woff:woff + 64],
                                        final_pad[c][:, oy:oy + 16, ox:ox + 32],
                                        start=(i == 0), stop=(i == 17),
                                        tile_position=(0, 64 * half))
                                    i += 1
                        nc.scalar.activation(
                            out_sb[ct_out][:, 16 * ch:16 * ch + 16, :],
                            pso[:].rearrange("p (a b) -> p a b", b=32),
                            AF.Relu, bias=biasb[ct_out][:])
                        (nc.sync if ch == 0 else nc.scalar).dma_start(
                            out_d[128 * ct_out:128 * (ct_out + 1),
                                  16 * ch:16 * ch + 16, :],
                            out_sb[ct_out][:, 16 * ch:16 * ch + 16, :])


def _get_nc():
    if "nc" not in _CACHE:
        _CACHE["nc"] = build_program()
    return _CACHE["nc"]


def kernel(foreground, mask, background, conv_w, conv_b):
    nc = _get_nc()
    fg = np.ascontiguousarray(foreground, dtype=np.float32).reshape(
        8, 2, 128, 32, 32).astype(ml_dtypes.bfloat16)
    bg = np.ascontiguousarray(background, dtype=np.float32).reshape(
        8, 2, 128, 32, 32).astype(ml_dtypes.bfloat16)
    maskrow = np.ascontiguousarray(mask.reshape(1, 1024), dtype=np.float32)
    # conv_w [4,64,256,3,3] -> [c, g, dy, dx, o] -> [2, 128, 2304] bf16
    wre = np.ascontiguousarray(
        conv_w.astype(np.float32).transpose(2, 0, 3, 4, 1).reshape(2, 128, 2304)
    ).astype(ml_dtypes.bfloat16)
    bias = np.ascontiguousarray(conv_b.astype(np.float32).reshape(256, 1))
    in_maps = [
        {"fg": fg[i], "bg": bg[i], "maskrow": maskrow, "wconv": wre, "bias": bias}
        for i in range(8)
    ]
    res = run_bass_kernel_spmd(nc, in_maps, list(range(8)))
    return np.stack([res.results[i]["out"] for i in range(8)], axis=0)


if __name__ == "__main__":
    build_program()
    print("build ok")



# revision 19
# speedup vs baseline: 1.3408x; 1.3408x over previous
"""Trainium2 Bass kernel for ContextualAttentionModule.

Data-parallel over batch: 8 samples -> 8 NeuronCores, one sample per core.
Per-core pipeline (C=256, H=W=32, L=1024 patches):
  scores  = <fg_patch(p), bg_patch(l)> via fp8e4 DoubleRow matmuls, hi-term
            only (the recovered/attention branch carries ~3% of output
            energy, so fp8 rounding there is far under the error budget)
  norm    = rsqrt(3x3-boxsum(colsum bg_masked^2) + 2304*eps^2), bf16 boxsum
  prop    = 3x3 window-sum of scores + eps*boxbox(colsum fg)  (DVE, bf16)
  E       = exp(prop * rn) in bf16 (no max-subtract)
  D, s2   = interleaved per-block [1,512] PE accumulations over E (lag 2)
  T8      = fp8(E * (1/D) * min(rn*2048, 256))  -- the 2^11 gain keeps the
            attn operand in fp8's normal range; undone exactly via the
            mask/(9*2048) constant tile.  1/D read directly from PSUM.
  recov   = fp8 DoubleRow tconv: t-block pairs contract in one matmul
            (lhs = transposed masked-bg banks quantized to fp8, rhs = T8)
  final   = (recov*mask/(9*2048)*2048 + eps*box(s2/D))*... + fg*(1-mask)
  out     = concat_g relu(dilated_conv_r(final) + b)  bf16 weights
"""

import numpy as np
import ml_dtypes

import concourse.bass as bass
import concourse.tile as tile
from concourse import bacc, mybir
from concourse.bass_utils import run_bass_kernel_spmd
from concourse.masks import make_identity

F32 = mybir.dt.float32
F32R = mybir.dt.float32r
BF16 = mybir.dt.bfloat16
F8 = mybir.dt.float8e4
U8 = mybir.dt.uint8
U16 = mybir.dt.uint16
AF = mybir.ActivationFunctionType
ALU = mybir.AluOpType
DR = mybir.MatmulPerfMode.DoubleRow

EPS = 1e-7
TS = 2048.0          # power-of-2 gain on the attn operand for fp8 range
RATES = (1, 2, 4, 8)
OFFS = [(dy, dx) for dy in range(3) for dx in range(3)]
# dy=1 taps first so the start=True matmul covers the full psum tile
TAPS = [(1, 0), (1, 1), (1, 2), (0, 0), (0, 1), (0, 2), (2, 0), (2, 1), (2, 2)]

_CACHE = {}


def _ring_zero(nc, buf, n=34, eng=None):
    """Zero only the 1-wide border ring of a [P, n, n] padded buffer."""
    eng = eng or nc.vector
    bc = U16 if buf.dtype == BF16 else F32
    eng.memset(buf[:, 0:n:n - 1, :].bitcast(bc), 0)
    eng.memset(buf[:, 1:n - 1, 0:n:n - 1].bitcast(bc), 0)


def _ring_zero_pair(nc, buf, eng=None):
    """Ring-zero both j-halves of a [P, 2, 34, 34] paired buffer."""
    eng = eng or nc.vector
    bc = U16 if buf.dtype == BF16 else U8
    eng.memset(buf[:, :, 0:34:33, :].bitcast(bc), 0)
    eng.memset(buf[:, :, 1:33, 0:34:33].bitcast(bc), 0)


def _boxsum(nc, scr, src_pad, dst_flat, eng=None):
    """3x3 SAME window sum: [1,34,34] ring-zero padded -> [1,32,32] flat."""
    eng = eng or nc.vector
    eng.tensor_tensor(scr[:, 1:33, 1:33], src_pad[:, 1:33, 0:32],
                      src_pad[:, 1:33, 1:33], ALU.add)
    eng.tensor_tensor(scr[:, 1:33, 1:33], scr[:, 1:33, 1:33],
                      src_pad[:, 1:33, 2:34], ALU.add)
    eng.tensor_tensor(dst_flat[:], scr[:, 0:32, 1:33],
                      scr[:, 1:33, 1:33], ALU.add)
    eng.tensor_tensor(dst_flat[:], dst_flat[:], scr[:, 2:34, 1:33], ALU.add)


def build_program():
    nc = bacc.Bacc()
    fg_d = nc.declare_dram_parameter("fg", [2, 128, 32, 32], BF16, isOutput=False)
    bg_d = nc.declare_dram_parameter("bg", [2, 128, 32, 32], BF16, isOutput=False)
    mask_d = nc.declare_dram_parameter("maskrow", [1, 1024], F32, isOutput=False)
    w_d = nc.declare_dram_parameter("wconv", [2, 128, 2304], BF16, isOutput=False)
    b_d = nc.declare_dram_parameter("bias", [256, 1], F32, isOutput=False)
    out_d = nc.declare_dram_parameter("out", [256, 32, 32], F32, isOutput=True)

    with tile.TileContext(nc) as tc:
        _emit(nc, tc, fg_d, bg_d, mask_d, w_d, b_d, out_d)
    nc.compile()
    return nc


def _emit(nc, tc, fg_d, bg_d, mask_d, w_d, b_d, out_d):
    with tc.tile_pool(name="main", bufs=1) as main:
        # ---------------- long-lived tiles ----------------
        fflat = main.tile([128, 2, 32, 32], BF16, name="fflat")
        invmaskb = main.tile([128, 32, 32], F32, name="invmaskb")
        maskb9 = main.tile([128, 32, 32], F32, name="maskb9")
        maskb9s = main.tile([128, 32, 32], F32, name="maskb9s")
        ones_col = main.tile([128, 1], F32R, name="ones_col")
        idR = main.tile([128, 128], BF16, name="idR")
        rncol = main.tile([128, 8], F32, name="rncol")
        rnccol = main.tile([128, 8], F32, name="rnccol")
        ercol = main.tile([128, 8], BF16, name="ercol")
        onesB = main.tile([128, 1], BF16, name="onesB")
        Gb = main.tile([128, 32, 32], BF16, name="Gb")
        msrow = main.tile([1, 1024], F32, name="msrow")
        Ap = [main.tile([128, 2, 34, 34], BF16, name=f"Ap{q}") for q in range(4)]
        A = [Ap[t // 2][:, t % 2] for t in range(8)]
        T8p = [main.tile([128, 2, 34, 34], F8, name=f"T8p{q}") for q in range(4)]
        W = [main.tile([128, 34, 34], BF16, name=f"W{i}") for i in range(2)]

        with tc.tile_pool(name="work", bufs=1) as work:
            # masked-bg x-shift variants: tile v holds value x[w + v - 1]
            # (0 at edges); rows padded (34) so dy comes from row offsets.
            bgb = [work.tile([128, 2, 34, 32], BF16, name=f"bgb{v}")
                   for v in range(3)]
            bg8h = [work.tile([128, 2, 34, 32], F8, name=f"bg8h{v}")
                    for v in range(3)]
            fg8h = [work.tile([128, 2, 32, 32], F8, name=f"fg8h{v}")
                    for v in range(3)]
            bgT8 = [work.tile([128, 2, 2304], F8, name=f"bgT8{q}")
                    for q in range(4)]
            rdrow = work.tile([1, 1024], F32, name="rdrow")
            s2row = work.tile([1, 1024], F32, name="s2row")

            stage_cm = tc.tile_pool(name="stage", bufs=1)
            stage = stage_cm.__enter__()
            ps_acc_cm = tc.tile_pool(name="ps_acc", bufs=1, space="PSUM")
            ps_acc = ps_acc_cm.__enter__()
            ps_sc_cm = tc.tile_pool(name="ps_sc", bufs=2, space="PSUM")
            ps_sc = ps_sc_cm.__enter__()
            ps_tra_cm = tc.tile_pool(name="ps_tra", bufs=2, space="PSUM")
            ps_tra = ps_tra_cm.__enter__()

            bq = stage.tile([128, 2, 32, 32], BF16, name="bq")
            q2 = stage.tile([128, 2, 32, 32], F32R, name="q2")
            onesf0 = stage.tile([128, 1], F32, name="onesf0")

            # ---------- phase 0: loads + edge zeros + quantize ----------
            nc.sync.dma_start(msrow[:], mask_d[:])
            nc.gpsimd.dma_start(bq[:, 0], bg_d[0])
            nc.sync.dma_start(bq[:, 1], bg_d[1])
            nc.scalar.dma_start(fflat[:, 0], fg_d[0])
            nc.sync.dma_start(fflat[:, 1], fg_d[1])

            nc.gpsimd.memset(onesf0[:], 1.0)
            # preload the Sqrt activation table (phase1 uses it first);
            # the Exp table is loaded once, later, via a dummy.
            nc.scalar.activation(onesf0[0:1], onesf0[0:1], AF.Sqrt)
            nc.gpsimd.memset(onesf0[:], 1.0)

            # load-independent zero-fills while DMAs fly
            onesrowR = stage.tile([1, 128], F32R, name="onesrowR")
            nc.vector.memset(onesrowR[:].bitcast(F32), 1.0)
            idf = stage.tile([128, 128], F32, name="idf")
            make_identity(nc, idf[:])
            nc.gpsimd.tensor_copy(idR[:], idf[:])
            nc.vector.tensor_copy(ones_col[:], onesf0[:])
            nc.gpsimd.tensor_copy(onesB[:], onesf0[:])
            for q in range(4):
                _ring_zero_pair(nc, Ap[q], eng=(nc.vector if q % 2 else nc.gpsimd))
                _ring_zero_pair(nc, T8p[q], eng=(nc.gpsimd if q % 2 else nc.vector))
            for w in W:
                _ring_zero(nc, w, eng=nc.gpsimd)
            for lst, edge in ((bgb, U16), (bg8h, U8)):
                nc.vector.memset(lst[0][:, :, 0:34:33, :].bitcast(edge), 0)
                nc.gpsimd.memset(lst[1][:, :, 0:34:33, :].bitcast(edge), 0)
                nc.vector.memset(lst[2][:, :, 0:34:33, :].bitcast(edge), 0)
                nc.vector.memset(lst[0][:, :, 1:33, 0:1].bitcast(edge), 0)
                nc.gpsimd.memset(lst[2][:, :, 1:33, 31:32].bitcast(edge), 0)
            nc.vector.memset(fg8h[0][:, :, :, 0:1].bitcast(U8), 0)
            nc.gpsimd.memset(fg8h[2][:, :, :, 31:32].bitcast(U8), 0)

            # mask broadcast via PE ones-outer-product
            msrowR = stage.tile([1, 1024], F32R, name="msrowR")
            nc.vector.tensor_copy(msrowR[:], msrow[:])
            invmaskB = stage.tile([128, 32, 32], BF16, name="invmaskB")
            for h in range(2):
                psb = ps_sc.tile([128, 16, 32], F32, name="psb", tag="psc")
                nc.tensor.matmul(
                    psb[:].rearrange("p a b -> p (a b)"), onesrowR[:],
                    msrowR[:, 512 * h:512 * (h + 1)], start=True, stop=True)
                nc.vector.tensor_scalar(
                    out=invmaskB[:, 16 * h:16 * (h + 1), :], in0=psb[:],
                    scalar1=-1.0, scalar2=1.0, op0=ALU.mult, op1=ALU.add)

            # q2 = bg^2 (unmasked; per-pixel mask applied on the ssq row)
            nc.scalar.activation(q2[:, 0], bq[:, 0], AF.Square)
            nc.scalar.activation(q2[:, 1], bq[:, 1], AF.Square)

            # masked bg x-variants, bf16 (transpose source) -- STT from raw
            # loads; fp8 copies for the scores lhs banks.
            SHIFT = {0: (1, 32, 0, 31), 1: (0, 32, 0, 32), 2: (0, 31, 1, 32)}
            k = 0
            for v in range(3):
                c0, c1, s0, s1 = SHIFT[v]
                for cb in range(2):
                    eng = nc.vector if k % 2 == 0 else nc.gpsimd
                    eng.tensor_tensor(
                        bgb[v][:, cb, 1:33, c0:c1], bq[:, cb, :, s0:s1],
                        invmaskB[:, :, s0:s1], ALU.mult)
                    k += 1
            for v in range(3):
                for cb in range(2):
                    eng = (nc.scalar, nc.vector, nc.gpsimd)[(v + cb) % 3]
                    if eng is nc.scalar:
                        nc.scalar.copy(bg8h[v][:, cb], bgb[v][:, cb])
                    else:
                        eng.tensor_copy(bg8h[v][:, cb], bgb[v][:, cb])

            # fg fp8: center on Act (split cb), dx variants DVE/Pool
            nc.scalar.copy(fg8h[1][:, 0], fflat[:, 0])
            nc.scalar.copy(fg8h[1][:, 1], fflat[:, 1])
            nc.vector.tensor_copy(fg8h[0][:, :, :, 1:32], fflat[:, :, :, 0:31])
            nc.gpsimd.tensor_copy(
                fg8h[2][:, :, :, 0:31].bitcast(U8),
                fg8h[1][:, :, :, 1:32].bitcast(U8))

            # ---------- phase 1: patch norms + Gb (emitted before scores
            # so its PE matmuls run while the quantize chain finishes) ----
            rpad = stage.tile([1, 34, 34], BF16, name="rpad")
            rscr = stage.tile([1, 34, 34], BF16, name="rscr")
            _ring_zero(nc, rpad, eng=nc.gpsimd)
            _ring_zero(nc, rscr, eng=nc.gpsimd)
            invrow = stage.tile([1, 1024], F32, name="invrow")
            nc.vector.tensor_scalar(out=invrow[:], in0=msrow[:],
                                    scalar1=-1.0, scalar2=1.0,
                                    op0=ALU.mult, op1=ALU.add)
            sqrow = stage.tile([1, 1024], F32, name="sqrow")
            for ch in range(2):
                pr = ps_sc.tile([1, 512], F32, name="pr", tag="psc")
                for cb in range(2):
                    nc.tensor.matmul(
                        pr[:], ones_col[:],
                        q2[:, cb, 16 * ch:16 * ch + 16, :],
                        start=(cb == 0), stop=(cb == 1))
                # mask per-pixel on the row, then stash into padded grid
                nc.vector.tensor_tensor(
                    sqrow[:, 512 * ch:512 * (ch + 1)], pr[:],
                    invrow[:, 512 * ch:512 * (ch + 1)], ALU.mult)
                nc.scalar.copy(
                    rpad[:, 1 + 16 * ch:17 + 16 * ch, 1:33],
                    sqrow[:, 512 * ch:512 * (ch + 1)].rearrange(
                        "o (a b) -> o a b", b=32))
            ssqrow = stage.tile([1, 32, 32], BF16, name="ssqrow")
            _boxsum(nc, rscr, rpad, ssqrow)
            urow = stage.tile([1, 1024], F32, name="urow")
            nc.vector.tensor_scalar_add(
                urow[:], ssqrow[:].rearrange("o a b -> o (a b)"),
                2304.0 * EPS * EPS)
            nc.scalar.activation(urow[:], urow[:], AF.Sqrt)
            nc.vector.reciprocal(urow[:], urow[:])
            # rncol[p, t] = rnrow[128t + p]: broadcast + diag extract
            Bb = stage.tile([128, 1024], F32, name="Bb")
            nc.gpsimd.partition_broadcast(Bb[:], urow[:])
            junk = stage.tile([128, 128], F32, name="junk")
            for t in range(8):
                nc.vector.scalar_tensor_tensor(
                    out=junk[:], in0=Bb[:, 128 * t:128 * (t + 1)],
                    scalar=1.0, in1=idf[:], op0=ALU.mult, op1=ALU.mult,
                    accum_out=rncol[:, t:t + 1])
            nc.vector.tensor_scalar(out=rnccol[:], in0=rncol[:], scalar1=TS,
                                    scalar2=256.0, op0=ALU.mult, op1=ALU.min)
            nc.vector.tensor_scalar_mul(ercol[:], rncol[:], EPS)

            # boxg: G = 3x3 box of channel-sum(fg); Gb = eps*box(G) bcast
            for ch in range(2):
                prg = ps_sc.tile([1, 512], F32, name="prg", tag="psc")
                for cb in range(2):
                    nc.tensor.matmul(
                        prg[:], onesB[:],
                        fflat[:, cb, 16 * ch:16 * ch + 16, :],
                        start=(cb == 0), stop=(cb == 1))
                nc.scalar.copy(rpad[:, 1 + 16 * ch:17 + 16 * ch, 1:33], prg[:])
            _boxsum(nc, rscr, rpad, ssqrow)
            nc.vector.tensor_copy(rpad[:, 1:33, 1:33], ssqrow[:])
            bbrow = stage.tile([1, 32, 32], BF16, name="bbrow")
            _boxsum(nc, rscr, rpad, bbrow)
            ebrowB = stage.tile([1, 1024], BF16, name="ebrowB")
            nc.vector.tensor_scalar_mul(
                ebrowB[:], bbrow[:].rearrange("o a b -> o (a b)"), EPS)
            nc.gpsimd.partition_broadcast(
                Gb.rearrange("p a b -> p (a b)"), ebrowB[:])
            # load the Exp table now (single switch, hidden under scores)
            nc.scalar.activation(onesf0[0:1], onesf0[0:1], AF.Exp)

            # mask constants (off critical path)
            nc.gpsimd.tensor_copy(invmaskb[:], invmaskB[:])
            nc.gpsimd.tensor_scalar(
                out=maskb9[:], in0=invmaskB[:], scalar1=-1.0 / 9.0,
                scalar2=1.0 / 9.0, op0=ALU.mult, op1=ALU.add)
            nc.gpsimd.tensor_scalar(
                out=maskb9s[:], in0=invmaskB[:], scalar1=-1.0 / (9.0 * TS),
                scalar2=1.0 / (9.0 * TS), op0=ALU.mult, op1=ALU.add)

            # ---------- scores loop (fp8 DR, hi only) ----------
            # row-accumulators pair up in PSUM banks via partition offsets
            accD = ps_acc.tile([128, 512], F32, name="accD")
            accS = ps_acc.tile([128, 512], F32, name="accS")
            dAcc = [accD[64 * c:64 * c + 1, :] for c in range(2)]
            s2Acc = [accS[64 * c:64 * c + 1, :] for c in range(2)]

            def emit_scores(t, ch):
                psc = ps_sc.tile([128, 16, 32], F32, name="psc", tag="psc")
                for i, (dy, dx) in enumerate(TAPS):
                    fr = 16 * ch + dy - 1
                    r0, r1 = max(0, -fr), min(16, 32 - fr)
                    nc.tensor.matmul(
                        psc[:, r0:r1, :],
                        bg8h[dx][:, :, 4 * t + dy:4 * t + dy + 4, :],
                        fg8h[dx][:, :, fr + r0:fr + r1, :],
                        start=(i == 0), stop=(i == 8), perf_mode=DR)
                half = A[t][:, 1 + 16 * ch:17 + 16 * ch, 1:33]
                if ch == 0:
                    nc.scalar.copy(half, psc[:])
                else:
                    nc.vector.tensor_copy(half, psc[:])

            hp_cm = tc.tile_pool(name="hp", bufs=2)
            hp = hp_cm.__enter__()

            def emit_boxsum_exp(t):
                w = W[t % 2]
                nc.vector.tensor_tensor(
                    w[:, 1:33, 1:33], A[t][:, 1:33, 0:32],
                    A[t][:, 1:33, 1:33], ALU.add)
                nc.vector.tensor_tensor(
                    w[:, 1:33, 1:33], w[:, 1:33, 1:33],
                    A[t][:, 1:33, 2:34], ALU.add)
                H = hp.tile([128, 32, 32], BF16, name="H", tag="H")
                nc.vector.tensor_tensor(
                    H[:], w[:, 0:32, 1:33], w[:, 1:33, 1:33], ALU.add)
                nc.vector.tensor_tensor(H[:], H[:], w[:, 2:34, 1:33], ALU.add)
                nc.vector.tensor_tensor(H[:], H[:], Gb[:], ALU.add)
                nc.scalar.activation(A[t][:, 1:33, 1:33], H[:],
                                     AF.Exp, scale=rncol[:, t:t + 1])

            def emit_d_s2(t):
                for ch in range(2):
                    av = A[t][:, 1 + 16 * ch:17 + 16 * ch, 1:33]
                    nc.tensor.matmul(dAcc[ch][:], onesB[:], av,
                                     start=(t == 0), stop=(t == 7))
                    nc.tensor.matmul(s2Acc[ch][:], ercol[:, t:t + 1],
                                     av, start=(t == 0), stop=(t == 7))

            def build_bgT8(q):
                # transpose (j, cb, d) 128x128 blocks of shifted masked bg
                # into bf16 psum, 4 at a time, then fp8-quantize to SBUF.
                blocks = [(j, cb, d) for j in range(2) for cb in range(2)
                          for d in range(9)]
                for g in range(9):
                    chunk = blocks[4 * g:4 * g + 4]
                    ptr = ps_tra.tile([128, 512], BF16, name="ptr", tag="ptr")
                    for bi, (j, cb, d) in enumerate(chunk):
                        dy, dx = OFFS[d]
                        t = 2 * q + j
                        nc.tensor.transpose(
                            ptr[:, 128 * bi:128 * (bi + 1)],
                            bgb[dx][:, cb, 4 * t + dy:4 * t + dy + 4, :]
                            .rearrange("p a b -> p (a b)"),
                            idR[:])
                    dst = bgT8[q].rearrange("p j c -> p (j c)")[
                        :, 512 * g:512 * g + 512]
                    if g % 2 == 0:
                        nc.scalar.copy(dst, ptr[:])
                    else:
                        nc.vector.tensor_copy(dst, ptr[:])

            for t in range(8):
                emit_scores(t, 0)
                if t >= 2:
                    emit_d_s2(t - 2)
                emit_scores(t, 1)
                if t % 2 == 1:
                    build_bgT8(t // 2)
                emit_boxsum_exp(t)
            emit_d_s2(6)
            emit_d_s2(7)
            hp_cm.__exit__(None, None, None)
            ps_tra_cm.__exit__(None, None, None)
            ps_sc_cm.__exit__(None, None, None)

            # ---------- softmax denom + T8 + tconv ----------
            ps_db_cm = tc.tile_pool(name="ps_db", bufs=1, space="PSUM")
            ps_db = ps_db_cm.__enter__()
            psd = [ps_db.tile([128, 512], F32, name=f"psd{c}") for c in range(2)]
            rdrowR = stage.tile([1, 1024], F32R, name="rdrowR")
            for ch in range(2):
                nc.vector.reciprocal(
                    rdrow[:, 512 * ch:512 * (ch + 1)], dAcc[ch][:])
                nc.vector.tensor_copy(rdrowR[:, 512 * ch:512 * (ch + 1)],
                                      rdrow[:, 512 * ch:512 * (ch + 1)])
                nc.tensor.matmul(psd[ch][:], onesrowR[:],
                                 rdrowR[:, 512 * ch:512 * (ch + 1)],
                                 start=True, stop=True)
                nc.vector.tensor_tensor(
                    s2row[:, 512 * ch:512 * (ch + 1)], s2Acc[ch][:],
                    rdrow[:, 512 * ch:512 * (ch + 1)], ALU.mult)

            Db = stage.tile([128, 32, 32], BF16, name="Db")
            for ch in range(2):
                nc.vector.tensor_copy(
                    Db[:, 16 * ch:16 * ch + 16, :],
                    psd[ch][:].rearrange("p (a b) -> p a b", b=32))

            def emit_T8(t):
                q, j = t // 2, t % 2
                for ch in range(2):
                    nc.vector.scalar_tensor_tensor(
                        out=T8p[q][:, j, 1 + 16 * ch:17 + 16 * ch, 1:33],
                        in0=A[t][:, 1 + 16 * ch:17 + 16 * ch, 1:33],
                        scalar=rnccol[:, t:t + 1],
                        in1=Db[:, 16 * ch:16 * ch + 16, :],
                        op0=ALU.mult, op1=ALU.mult)

            with tc.tile_pool(name="ps_rec", bufs=1, space="PSUM") as psrec:
                prec = [[psrec.tile([128, 512], F32, name=f"prec{c}_{ch}")
                         for ch in range(2)] for c in range(2)]

                def emit_tconv(q):
                    for cb in range(2):
                        for ch in range(2):
                            for d, (dy, dx) in enumerate(OFFS):
                                z0 = 16 * ch + 2 - dy
                                x0 = 2 - dx
                                nc.tensor.matmul(
                                    prec[cb][ch][:],
                                    bgT8[q][:, :, 128 * (9 * cb + d):
                                            128 * (9 * cb + d + 1)],
                                    T8p[q][:, :, z0:z0 + 16, x0:x0 + 32],
                                    start=(q == 0 and d == 0),
                                    stop=(q == 3 and d == 8), perf_mode=DR)

                for t in range(8):
                    emit_T8(t)
                for q in range(4):
                    emit_tconv(q)

                # s2 path: s2b = eps*box(s2row/D) * mask/9, broadcast
                s2pad = W[0][0:1]
                s2scr = W[1][0:1]
                _ring_zero(nc, s2pad, eng=nc.gpsimd)
                _ring_zero(nc, s2scr, eng=nc.gpsimd)
                nc.gpsimd.tensor_copy(
                    s2pad[:, 1:33, 1:33],
                    s2row[:].rearrange("o (a b) -> o a b", b=32))
                boxs2 = stage.tile([1, 32, 32], BF16, name="boxs2")
                _boxsum(nc, s2scr, s2pad, boxs2, eng=nc.gpsimd)
                nc.vector.tensor_scalar_mul(
                    s2row[:], boxs2[:].rearrange("o a b -> o (a b)"), EPS)
                s2b = stage.tile([128, 32, 32], F32, name="s2b")
                nc.gpsimd.partition_broadcast(
                    s2b.rearrange("p a b -> p (a b)"), s2row[:])
                nc.vector.tensor_tensor(s2b[:], s2b[:], maskb9[:], ALU.mult)

                # final = prec*mask/(9*TS) + eps*boxs2*mask/9 + fg*(1-mask)
                final_pad = [main.tile([128, 48, 48], BF16,
                                       name=f"final_pad{c}") for c in range(2)]
                for c in range(2):
                    nc.vector.memset(final_pad[c][:, 0:8, :].bitcast(U16), 0)
                    nc.vector.memset(final_pad[c][:, 40:48, :].bitcast(U16), 0)
                    nc.gpsimd.memset(
                        final_pad[c][:, 8:40, 0:8].bitcast(U16), 0)
                    nc.gpsimd.memset(
                        final_pad[c][:, 8:40, 40:48].bitcast(U16), 0)
                fscr = [stage.tile([128, 32, 32], F32, name=f"fscr{i}")
                        for i in range(2)]
                for cb in range(2):
                    for ch in range(2):
                        r0 = 16 * ch
                        nc.vector.tensor_tensor(
                            fscr[0][:, r0:r0 + 16, :], prec[cb][ch][:],
                            maskb9s[:, r0:r0 + 16, :], ALU.mult)
                    nc.vector.tensor_tensor(fscr[0][:], fscr[0][:],
                                            s2b[:], ALU.add)
                    nc.gpsimd.tensor_tensor(fscr[1][:], fflat[:, cb],
                                            invmaskb[:], ALU.mult)
                    nc.vector.tensor_tensor(
                        final_pad[cb][:, 8:40, 8:40],
                        fscr[0][:], fscr[1][:], ALU.add)
            ps_db_cm.__exit__(None, None, None)
            ps_acc_cm.__exit__(None, None, None)
            stage_cm.__exit__(None, None, None)
        # ---------- work pool closed ----------

        with tc.tile_pool(name="late", bufs=1) as late:
            # ---------- dilated convs (bf16) ----------
            wsb = [late.tile([128, 2304], BF16, name=f"wsb{c}") for c in range(2)]
            biasb = [late.tile([128, 1], F32, name=f"biasb{c}") for c in range(2)]
            for c in range(2):
                nc.scalar.dma_start(wsb[c][:], w_d[c])
                nc.scalar.dma_start(biasb[c][:], b_d[128 * c:128 * (c + 1)])
            out_sb = [late.tile([128, 32, 32], F32, name=f"out_sb{c}")
                      for c in range(2)]

            with tc.tile_pool(name="ps_o", bufs=3, space="PSUM") as pso_pool:
                for ct_out in range(2):
                    for ch in range(2):
                        pso = pso_pool.tile([128, 512], F32, name="pso",
                                            tag="pso")
                        for half in range(2):
                            g = 2 * ct_out + half
                            r = RATES[g]
                            i = 0
                            for c in range(2):
                                for d, (dy, dx) in enumerate(OFFS):
                                    oy = 8 + r * (dy - 1) + 16 * ch
                                    ox = 8 + r * (dx - 1)
                                    woff = 576 * g + 64 * (3 * dy + dx)
                                    nc.tensor.matmul(
                                        pso[64 * half:64 * half + 64, :],
                                        wsb[c][:, woff:woff + 64],
                                        final_pad[c][:, oy:oy + 16, ox:ox + 32],
                                        start=(i == 0), stop=(i == 17),
                                        tile_position=(0, 64 * half))
                                    i += 1
                        nc.scalar.activation(
                            out_sb[ct_out][:, 16 * ch:16 * ch + 16, :],
                            pso[:].rearrange("p (a b) -> p a b", b=32),
                            AF.Relu, bias=biasb[ct_out][:])
                        (nc.sync if ch == 0 else nc.scalar).dma_start(
                            out_d[128 * ct_out:128 * (ct_out + 1),
                                  16 * ch:16 * ch + 16, :],
                            out_sb[ct_out][:, 16 * ch:16 * ch + 16, :])


def _get_nc():
    if "nc" not in _CACHE:
        _CACHE["nc"] = build_program()
    return _CACHE["nc"]


def kernel(foreground, mask, background, conv_w, conv_b):
    nc = _get_nc()
    fg = np.ascontiguousarray(foreground, dtype=np.float32).reshape(
        8, 2, 128, 32, 32).astype(ml_dtypes.bfloat16)
    bg = np.ascontiguousarray(background, dtype=np.float32).reshape(
        8, 2, 128, 32, 32).astype(ml_dtypes.bfloat16)
    maskrow = np.ascontiguousarray(mask.reshape(1, 1024), dtype=np.float32)
    # conv_w [4,64,256,3,3] -> [c, g, dy, dx, o] -> [2, 128, 2304] bf16
    wre = np.ascontiguousarray(
        conv_w.astype(np.float32).transpose(2, 0, 3, 4, 1).reshape(2, 128, 2304)
    ).astype(ml_dtypes.bfloat16)
    bias = np.ascontiguousarray(conv_b.astype(np.float32).reshape(256, 1))
    in_maps = [
        {"fg": fg[i], "bg": bg[i], "maskrow": maskrow, "wconv": wre, "bias": bias}
        for i in range(8)
    ]
    res = run_bass_kernel_spmd(nc, in_maps, list(range(8)))
    return np.stack([res.results[i]["out"] for i in range(8)], axis=0)


if __name__ == "__main__":
    build_program()
    print("build ok")


# revision 29
# speedup vs baseline: 1.4792x; 1.1032x over previous
"""Trainium2 Bass kernel for ContextualAttentionModule.

Data-parallel over batch: 8 samples -> 8 NeuronCores, one sample per core.
Per-core pipeline (C=256, H=W=32, L=1024 patches):
  scores  = <fg_patch(p), bg_patch(l)> via fp8e4 DoubleRow matmuls, hi-term
            only (the recovered/attention branch carries ~3% of output
            energy, so fp8 rounding there is far under the error budget)
  norm    = rsqrt(3x3-boxsum(colsum bg_masked^2) + 2304*eps^2), bf16 boxsum
  prop    = 3x3 window-sum of scores + eps*boxbox(colsum fg)  (DVE, bf16)
  E       = exp(prop * rn) in bf16 (no max-subtract)
  D, s2   = interleaved per-block [1,512] PE accumulations over E (lag 2)
  T8      = fp8(E * (1/D) * min(rn*2048, 256))  -- the 2^11 gain keeps the
            attn operand in fp8's normal range; undone exactly via the
            mask/(9*2048) constant tile.  1/D read directly from PSUM.
  recov   = fp8 DoubleRow tconv: t-block pairs contract in one matmul
            (lhs = transposed masked-bg banks quantized to fp8, rhs = T8)
  final   = (recov*mask/(9*2048)*2048 + eps*box(s2/D))*... + fg*(1-mask)
  out     = concat_g relu(dilated_conv_r(final) + b)  bf16 weights
"""

import numpy as np
import ml_dtypes

import concourse.bass as bass
import concourse.tile as tile
from concourse import bacc, mybir
from concourse.bass_utils import run_bass_kernel_spmd
from concourse.masks import make_identity

F32 = mybir.dt.float32
F32R = mybir.dt.float32r
BF16 = mybir.dt.bfloat16
F8 = mybir.dt.float8e4
U8 = mybir.dt.uint8
U16 = mybir.dt.uint16
AF = mybir.ActivationFunctionType
ALU = mybir.AluOpType
DR = mybir.MatmulPerfMode.DoubleRow

EPS = 1e-7
TS = 2048.0          # power-of-2 gain on the attn operand for fp8 range
RATES = (1, 2, 4, 8)
OFFS = [(dy, dx) for dy in range(3) for dx in range(3)]
# dy=1 taps first so the start=True matmul covers the full psum tile
TAPS = [(1, 0), (1, 1), (1, 2), (0, 0), (0, 1), (0, 2), (2, 0), (2, 1), (2, 2)]

_CACHE = {}


def _ring_zero(nc, buf, n=34, eng=None):
    """Zero only the 1-wide border ring of a [P, n, n] padded buffer."""
    eng = eng or nc.vector
    bc = U16 if buf.dtype == BF16 else F32
    eng.memset(buf[:, 0:n:n - 1, :].bitcast(bc), 0)
    eng.memset(buf[:, 1:n - 1, 0:n:n - 1].bitcast(bc), 0)


def _ring_zero_pair(nc, buf, eng=None):
    """Ring-zero both j-halves of a [P, 2, 34, 34] paired buffer."""
    eng = eng or nc.vector
    bc = U16 if buf.dtype == BF16 else U8
    eng.memset(buf[:, :, 0:34:33, :].bitcast(bc), 0)
    eng.memset(buf[:, :, 1:33, 0:34:33].bitcast(bc), 0)


def _boxsum(nc, scr, src_pad, dst_flat, eng=None):
    """3x3 SAME window sum: [1,34,34] ring-zero padded -> [1,32,32] flat."""
    eng = eng or nc.vector
    eng.tensor_tensor(scr[:, 1:33, 1:33], src_pad[:, 1:33, 0:32],
                      src_pad[:, 1:33, 1:33], ALU.add)
    eng.tensor_tensor(scr[:, 1:33, 1:33], scr[:, 1:33, 1:33],
                      src_pad[:, 1:33, 2:34], ALU.add)
    eng.tensor_tensor(dst_flat[:], scr[:, 0:32, 1:33],
                      scr[:, 1:33, 1:33], ALU.add)
    eng.tensor_tensor(dst_flat[:], dst_flat[:], scr[:, 2:34, 1:33], ALU.add)


def build_program():
    nc = bacc.Bacc()
    fg_d = nc.declare_dram_parameter("fg", [2, 128, 32, 32], BF16, isOutput=False)
    bg_d = nc.declare_dram_parameter("bg", [2, 128, 32, 32], BF16, isOutput=False)
    mask_d = nc.declare_dram_parameter("maskrow", [1, 1024], F32, isOutput=False)
    w_d = nc.declare_dram_parameter("wconv", [2, 128, 2304], BF16, isOutput=False)
    b_d = nc.declare_dram_parameter("bias", [256, 1], F32, isOutput=False)
    out_d = nc.declare_dram_parameter("out", [256, 32, 32], F32, isOutput=True)

    with tile.TileContext(nc) as tc:
        _emit(nc, tc, fg_d, bg_d, mask_d, w_d, b_d, out_d)
    nc.compile()
    return nc


def _emit(nc, tc, fg_d, bg_d, mask_d, w_d, b_d, out_d):
    with tc.tile_pool(name="main", bufs=1) as main:
        # ---------------- long-lived tiles ----------------
        fflat = main.tile([128, 2, 32, 32], BF16, name="fflat")
        invmaskb = main.tile([128, 32, 32], F32, name="invmaskb")
        maskb9 = main.tile([128, 32, 32], F32, name="maskb9")
        maskb9s = main.tile([128, 32, 32], F32, name="maskb9s")
        ones_col = main.tile([128, 1], F32R, name="ones_col")
        idR = main.tile([128, 128], BF16, name="idR")
        rncol = main.tile([128, 8], F32, name="rncol")
        rnccol = main.tile([128, 8], F32, name="rnccol")
        ercol = main.tile([128, 8], BF16, name="ercol")
        onesB = main.tile([128, 1], BF16, name="onesB")
        Gb = main.tile([128, 32, 32], BF16, name="Gb")
        msrow = main.tile([1, 1024], F32, name="msrow")
        Ap = [main.tile([128, 2, 34, 34], BF16, name=f"Ap{q}") for q in range(4)]
        A = [Ap[t // 2][:, t % 2] for t in range(8)]
        T8p = [main.tile([128, 2, 34, 34], F8, name=f"T8p{q}") for q in range(4)]
        W = [main.tile([128, 34, 34], BF16, name=f"W{i}") for i in range(2)]

        with tc.tile_pool(name="work", bufs=1) as work:
            # masked-bg x-shift variants: tile v holds value x[w + v - 1]
            # (0 at edges); rows padded (34) so dy comes from row offsets.
            bgb = [work.tile([128, 2, 34, 32], BF16, name=f"bgb{v}")
                   for v in range(3)]
            bg8h = [work.tile([128, 2, 34, 32], F8, name=f"bg8h{v}")
                    for v in range(3)]
            fg8h = [work.tile([128, 2, 32, 32], F8, name=f"fg8h{v}")
                    for v in range(3)]
            bgT8 = [work.tile([128, 2, 2304], F8, name=f"bgT8{q}")
                    for q in range(4)]
            rdrow = work.tile([1, 1024], F32, name="rdrow")
            s2row = work.tile([1, 1024], F32, name="s2row")

            stage_cm = tc.tile_pool(name="stage", bufs=1)
            stage = stage_cm.__enter__()
            ps_acc_cm = tc.tile_pool(name="ps_acc", bufs=1, space="PSUM")
            ps_acc = ps_acc_cm.__enter__()
            ps_sc_cm = tc.tile_pool(name="ps_sc", bufs=2, space="PSUM")
            ps_sc = ps_sc_cm.__enter__()
            ps_tra_cm = tc.tile_pool(name="ps_tra", bufs=2, space="PSUM")
            ps_tra = ps_tra_cm.__enter__()

            bq = stage.tile([128, 2, 32, 32], BF16, name="bq")
            q2 = stage.tile([128, 2, 32, 32], F32R, name="q2")
            onesf0 = stage.tile([128, 1], F32, name="onesf0")

            # ---------- phase 0: loads + edge zeros + quantize ----------
            nc.sync.dma_start(msrow[:], mask_d[:])
            nc.gpsimd.dma_start(bq[:, 0], bg_d[0])
            nc.sync.dma_start(bq[:, 1], bg_d[1])
            nc.scalar.dma_start(fflat[:, 0], fg_d[0])
            nc.sync.dma_start(fflat[:, 1], fg_d[1])

            nc.gpsimd.memset(onesf0[:], 1.0)
            # preload the ln+exp activation table once; every activation in
            # this kernel (Copy/Square/Ln/Exp/Relu) lives in it.
            nc.scalar.activation(onesf0[0:1], onesf0[0:1], AF.Ln)
            nc.scalar.activation(onesf0[0:1], onesf0[0:1], AF.Exp)
            nc.gpsimd.memset(onesf0[:], 1.0)

            # load-independent zero-fills while DMAs fly
            onesrowR = stage.tile([1, 128], F32R, name="onesrowR")
            nc.vector.memset(onesrowR[:].bitcast(F32), 1.0)
            idf = stage.tile([128, 128], F32, name="idf")
            make_identity(nc, idf[:])
            nc.gpsimd.tensor_copy(idR[:], idf[:])
            nc.vector.tensor_copy(ones_col[:], onesf0[:])
            nc.gpsimd.tensor_copy(onesB[:], onesf0[:])
            for q in range(4):
                _ring_zero_pair(nc, Ap[q], eng=(nc.vector if q % 2 else nc.gpsimd))
                _ring_zero_pair(nc, T8p[q], eng=(nc.gpsimd if q % 2 else nc.vector))
            for w in W:
                _ring_zero(nc, w, eng=nc.gpsimd)
            for lst, edge in ((bgb, U16), (bg8h, U8)):
                nc.vector.memset(lst[0][:, :, 0:34:33, :].bitcast(edge), 0)
                nc.gpsimd.memset(lst[1][:, :, 0:34:33, :].bitcast(edge), 0)
                nc.vector.memset(lst[2][:, :, 0:34:33, :].bitcast(edge), 0)
                nc.vector.memset(lst[0][:, :, 1:33, 0:1].bitcast(edge), 0)
                nc.gpsimd.memset(lst[2][:, :, 1:33, 31:32].bitcast(edge), 0)
            nc.vector.memset(fg8h[0][:, :, :, 0:1].bitcast(U8), 0)
            nc.gpsimd.memset(fg8h[2][:, :, :, 31:32].bitcast(U8), 0)

            # mask broadcast via PE ones-outer-product
            msrowR = stage.tile([1, 1024], F32R, name="msrowR")
            nc.vector.tensor_copy(msrowR[:], msrow[:])
            invmaskB = stage.tile([128, 32, 32], BF16, name="invmaskB")
            for h in range(2):
                psb = ps_sc.tile([128, 16, 32], F32, name="psb", tag="psc")
                nc.tensor.matmul(
                    psb[:].rearrange("p a b -> p (a b)"), onesrowR[:],
                    msrowR[:, 512 * h:512 * (h + 1)], start=True, stop=True)
                nc.vector.tensor_scalar(
                    out=invmaskB[:, 16 * h:16 * (h + 1), :], in0=psb[:],
                    scalar1=-1.0, scalar2=1.0, op0=ALU.mult, op1=ALU.add)

            # q2 = bg^2 (unmasked; per-pixel mask applied on the ssq row)
            nc.scalar.activation(q2[:, 0], bq[:, 0], AF.Square)
            nc.scalar.activation(q2[:, 1], bq[:, 1], AF.Square)

            # masked bg x-variants, bf16 (transpose source) -- STT from raw
            # loads; fp8 copies for the scores lhs banks.
            SHIFT = {0: (1, 32, 0, 31), 1: (0, 32, 0, 32), 2: (0, 31, 1, 32)}
            k = 0
            for v in range(3):
                c0, c1, s0, s1 = SHIFT[v]
                for cb in range(2):
                    eng = nc.vector if k % 2 == 0 else nc.gpsimd
                    eng.tensor_tensor(
                        bgb[v][:, cb, 1:33, c0:c1], bq[:, cb, :, s0:s1],
                        invmaskB[:, :, s0:s1], ALU.mult)
                    k += 1
            for v in range(3):
                for cb in range(2):
                    eng = (nc.scalar, nc.vector, nc.gpsimd)[(v + cb) % 3]
                    if eng is nc.scalar:
                        nc.scalar.copy(bg8h[v][:, cb], bgb[v][:, cb])
                    else:
                        eng.tensor_copy(bg8h[v][:, cb], bgb[v][:, cb])

            # fg fp8: center on Act (split cb), dx variants DVE/Pool
            nc.scalar.copy(fg8h[1][:, 0], fflat[:, 0])
            nc.scalar.copy(fg8h[1][:, 1], fflat[:, 1])
            nc.vector.tensor_copy(fg8h[0][:, :, :, 1:32], fflat[:, :, :, 0:31])
            nc.gpsimd.tensor_copy(
                fg8h[2][:, :, :, 0:31].bitcast(U8),
                fg8h[1][:, :, :, 1:32].bitcast(U8))

            # ---------- phase 1: patch norms + Gb (emitted before scores
            # so its PE matmuls run while the quantize chain finishes) ----
            rpad = stage.tile([1, 34, 34], BF16, name="rpad")
            rscr = stage.tile([1, 34, 34], BF16, name="rscr")
            _ring_zero(nc, rpad, eng=nc.gpsimd)
            _ring_zero(nc, rscr, eng=nc.gpsimd)
            invrow = stage.tile([1, 1024], F32, name="invrow")
            nc.vector.tensor_scalar(out=invrow[:], in0=msrow[:],
                                    scalar1=-1.0, scalar2=1.0,
                                    op0=ALU.mult, op1=ALU.add)
            sqrow = stage.tile([1, 1024], F32, name="sqrow")
            for ch in range(2):
                pr = ps_sc.tile([1, 512], F32, name="pr", tag="psc")
                for cb in range(2):
                    nc.tensor.matmul(
                        pr[:], ones_col[:],
                        q2[:, cb, 16 * ch:16 * ch + 16, :],
                        start=(cb == 0), stop=(cb == 1))
                # mask per-pixel on the row, then stash into padded grid
                nc.vector.tensor_tensor(
                    sqrow[:, 512 * ch:512 * (ch + 1)], pr[:],
                    invrow[:, 512 * ch:512 * (ch + 1)], ALU.mult)
                nc.scalar.copy(
                    rpad[:, 1 + 16 * ch:17 + 16 * ch, 1:33],
                    sqrow[:, 512 * ch:512 * (ch + 1)].rearrange(
                        "o (a b) -> o a b", b=32))
            ssqrow = stage.tile([1, 32, 32], BF16, name="ssqrow")
            _boxsum(nc, rscr, rpad, ssqrow)
            urow = stage.tile([1, 1024], F32, name="urow")
            nc.vector.tensor_scalar_add(
                urow[:], ssqrow[:].rearrange("o a b -> o (a b)"),
                2304.0 * EPS * EPS)
            # rn = 1/sqrt(u) = exp(-0.5 * ln u): stays in the ln+exp table
            nc.scalar.activation(urow[:], urow[:], AF.Ln)
            nc.scalar.activation(urow[:], urow[:], AF.Exp, scale=-0.5)
            # rncol[p, t] = rnrow[128t + p]: broadcast + diag extract
            Bb = stage.tile([128, 1024], F32, name="Bb")
            nc.gpsimd.partition_broadcast(Bb[:], urow[:])
            junk = stage.tile([128, 128], F32, name="junk")
            for t in range(8):
                nc.vector.scalar_tensor_tensor(
                    out=junk[:], in0=Bb[:, 128 * t:128 * (t + 1)],
                    scalar=1.0, in1=idf[:], op0=ALU.mult, op1=ALU.mult,
                    accum_out=rncol[:, t:t + 1])
            nc.vector.tensor_scalar(out=rnccol[:], in0=rncol[:], scalar1=TS,
                                    scalar2=256.0, op0=ALU.mult, op1=ALU.min)
            nc.vector.tensor_scalar_mul(ercol[:], rncol[:], EPS)

            # Gs = eps * 3x3-box of channel-sum(fg), broadcast.  Added into
            # A alongside the psc copy; the padded-ring boxsum over A then
            # supplies the outer box of the reference's eps*boxbox term.
            for ch in range(2):
                prg = ps_sc.tile([1, 512], F32, name="prg", tag="psc")
                for cb in range(2):
                    nc.tensor.matmul(
                        prg[:], onesB[:],
                        fflat[:, cb, 16 * ch:16 * ch + 16, :],
                        start=(cb == 0), stop=(cb == 1))
                nc.scalar.copy(rpad[:, 1 + 16 * ch:17 + 16 * ch, 1:33], prg[:])
            _boxsum(nc, rscr, rpad, ssqrow)
            ebrowB = stage.tile([1, 1024], BF16, name="ebrowB")
            nc.vector.tensor_scalar_mul(
                ebrowB[:], ssqrow[:].rearrange("o a b -> o (a b)"), EPS)
            nc.gpsimd.partition_broadcast(
                Gb.rearrange("p a b -> p (a b)"), ebrowB[:])

            # mask constants (off critical path)
            nc.gpsimd.tensor_copy(invmaskb[:], invmaskB[:])
            nc.gpsimd.tensor_scalar(
                out=maskb9[:], in0=invmaskB[:], scalar1=-1.0 / 9.0,
                scalar2=1.0 / 9.0, op0=ALU.mult, op1=ALU.add)
            nc.gpsimd.tensor_scalar(
                out=maskb9s[:], in0=invmaskB[:], scalar1=-1.0 / (9.0 * TS),
                scalar2=1.0 / (9.0 * TS), op0=ALU.mult, op1=ALU.add)

            # ---------- scores loop (fp8 DR, hi only) ----------
            # row-accumulators pair up in PSUM banks via partition offsets
            accD = ps_acc.tile([128, 512], F32, name="accD")
            accS = ps_acc.tile([128, 512], F32, name="accS")
            dAcc = [accD[64 * c:64 * c + 1, :] for c in range(2)]
            s2Acc = [accS[64 * c:64 * c + 1, :] for c in range(2)]

            def emit_scores(t, ch):
                psc = ps_sc.tile([128, 16, 32], F32, name="psc", tag="psc")
                for i, (dy, dx) in enumerate(TAPS):
                    fr = 16 * ch + dy - 1
                    r0, r1 = max(0, -fr), min(16, 32 - fr)
                    nc.tensor.matmul(
                        psc[:, r0:r1, :],
                        bg8h[dx][:, :, 4 * t + dy:4 * t + dy + 4, :],
                        fg8h[dx][:, :, fr + r0:fr + r1, :],
                        start=(i == 0), stop=(i == 8), perf_mode=DR)
                half = A[t][:, 1 + 16 * ch:17 + 16 * ch, 1:33]
                nc.vector.tensor_tensor(
                    half, psc[:], Gb[:, 16 * ch:16 * ch + 16, :], ALU.add)

            hp_cm = tc.tile_pool(name="hp", bufs=2)
            hp = hp_cm.__enter__()

            def emit_boxsum_exp(t):
                w = W[t % 2]
                nc.vector.tensor_tensor(
                    w[:, 1:33, 1:33], A[t][:, 1:33, 0:32],
                    A[t][:, 1:33, 1:33], ALU.add)
                nc.vector.tensor_tensor(
                    w[:, 1:33, 1:33], w[:, 1:33, 1:33],
                    A[t][:, 1:33, 2:34], ALU.add)
                H = hp.tile([128, 32, 32], BF16, name="H", tag="H")
                nc.vector.tensor_tensor(
                    H[:], w[:, 0:32, 1:33], w[:, 1:33, 1:33], ALU.add)
                nc.vector.tensor_tensor(H[:], H[:], w[:, 2:34, 1:33], ALU.add)
                nc.scalar.activation(A[t][:, 1:33, 1:33], H[:],
                                     AF.Exp, scale=rncol[:, t:t + 1])

            def emit_d_s2(t):
                for ch in range(2):
                    av = A[t][:, 1 + 16 * ch:17 + 16 * ch, 1:33]
                    nc.tensor.matmul(dAcc[ch][:], onesB[:], av,
                                     start=(t == 0), stop=(t == 7))
                    nc.tensor.matmul(s2Acc[ch][:], ercol[:, t:t + 1],
                                     av, start=(t == 0), stop=(t == 7))

            def build_bgT8(q):
                # transpose (j, cb, d) 128x128 blocks of shifted masked bg
                # into bf16 psum, 4 at a time, then fp8-quantize to SBUF.
                blocks = [(j, cb, d) for j in range(2) for cb in range(2)
                          for d in range(9)]
                for g in range(9):
                    chunk = blocks[4 * g:4 * g + 4]
                    ptr = ps_tra.tile([128, 512], BF16, name="ptr", tag="ptr")
                    for bi, (j, cb, d) in enumerate(chunk):
                        dy, dx = OFFS[d]
                        t = 2 * q + j
                        nc.tensor.transpose(
                            ptr[:, 128 * bi:128 * (bi + 1)],
                            bgb[dx][:, cb, 4 * t + dy:4 * t + dy + 4, :]
                            .rearrange("p a b -> p (a b)"),
                            idR[:])
                    dst = bgT8[q].rearrange("p j c -> p (j c)")[
                        :, 512 * g:512 * g + 512]
                    if g % 3 < 2:
                        nc.scalar.copy(dst, ptr[:])
                    else:
                        nc.vector.tensor_copy(dst, ptr[:])

            for t in range(8):
                emit_scores(t, 0)
                if t >= 3:
                    emit_d_s2(t - 3)
                emit_scores(t, 1)
                if t < 4:
                    build_bgT8(t)
                emit_boxsum_exp(t)
            emit_d_s2(5)
            emit_d_s2(6)
            emit_d_s2(7)
            hp_cm.__exit__(None, None, None)
            ps_tra_cm.__exit__(None, None, None)
            ps_sc_cm.__exit__(None, None, None)

            # ---------- softmax denom + T8 + tconv ----------
            ps_db_cm = tc.tile_pool(name="ps_db", bufs=1, space="PSUM")
            ps_db = ps_db_cm.__enter__()
            psd = [ps_db.tile([128, 512], F32, name=f"psd{c}") for c in range(2)]
            rdrowR = stage.tile([1, 1024], F32R, name="rdrowR")
            for ch in range(2):
                nc.vector.reciprocal(
                    rdrow[:, 512 * ch:512 * (ch + 1)], dAcc[ch][:])
                nc.vector.tensor_copy(rdrowR[:, 512 * ch:512 * (ch + 1)],
                                      rdrow[:, 512 * ch:512 * (ch + 1)])
                nc.tensor.matmul(psd[ch][:], onesrowR[:],
                                 rdrowR[:, 512 * ch:512 * (ch + 1)],
                                 start=True, stop=True)
                nc.vector.tensor_tensor(
                    s2row[:, 512 * ch:512 * (ch + 1)], s2Acc[ch][:],
                    rdrow[:, 512 * ch:512 * (ch + 1)], ALU.mult)

            Db = stage.tile([128, 32, 32], BF16, name="Db")
            for ch in range(2):
                nc.scalar.copy(
                    Db[:, 16 * ch:16 * ch + 16, :],
                    psd[ch][:].rearrange("p (a b) -> p a b", b=32))

            def emit_T8(t):
                q, j = t // 2, t % 2
                nc.vector.scalar_tensor_tensor(
                    out=T8p[q][:, j, 1:33, 1:33],
                    in0=A[t][:, 1:33, 1:33],
                    scalar=rnccol[:, t:t + 1],
                    in1=Db[:],
                    op0=ALU.mult, op1=ALU.mult)

            # s2 path early: runs on Pool while T8/tconv proceed
            s2pad = W[0][0:1]
            s2scr = W[1][0:1]
            _ring_zero(nc, s2pad, eng=nc.gpsimd)
            _ring_zero(nc, s2scr, eng=nc.gpsimd)
            nc.gpsimd.tensor_copy(
                s2pad[:, 1:33, 1:33],
                s2row[:].rearrange("o (a b) -> o a b", b=32))
            boxs2 = stage.tile([1, 32, 32], BF16, name="boxs2")
            _boxsum(nc, s2scr, s2pad, boxs2, eng=nc.gpsimd)
            nc.gpsimd.tensor_scalar_mul(
                s2row[:], boxs2[:].rearrange("o a b -> o (a b)"), EPS)
            s2b = stage.tile([128, 32, 32], F32, name="s2b")
            nc.gpsimd.partition_broadcast(
                s2b.rearrange("p a b -> p (a b)"), s2row[:])
            nc.gpsimd.tensor_tensor(s2b[:], s2b[:], maskb9[:], ALU.mult)

            with tc.tile_pool(name="ps_rec", bufs=1, space="PSUM") as psrec:
                prec = [[psrec.tile([128, 512], F32, name=f"prec{c}_{ch}")
                         for ch in range(2)] for c in range(2)]

                def emit_tconv(q):
                    for cb in range(2):
                        for ch in range(2):
                            for d, (dy, dx) in enumerate(OFFS):
                                z0 = 16 * ch + 2 - dy
                                x0 = 2 - dx
                                nc.tensor.matmul(
                                    prec[cb][ch][:],
                                    bgT8[q][:, :, 128 * (9 * cb + d):
                                            128 * (9 * cb + d + 1)],
                                    T8p[q][:, :, z0:z0 + 16, x0:x0 + 32],
                                    start=(q == 0 and d == 0),
                                    stop=(q == 3 and d == 8), perf_mode=DR)

                for t in range(8):
                    emit_T8(t)
                for q in range(4):
                    emit_tconv(q)

                # final = prec*mask/(9*TS) + eps*boxs2*mask/9 + fg*(1-mask)
                final_pad = [main.tile([128, 48, 48], BF16,
                                       name=f"final_pad{c}") for c in range(2)]
                for c in range(2):
                    nc.vector.memset(final_pad[c][:, 0:8, :].bitcast(U16), 0)
                    nc.vector.memset(final_pad[c][:, 40:48, :].bitcast(U16), 0)
                    nc.gpsimd.memset(
                        final_pad[c][:, 8:40, 0:8].bitcast(U16), 0)
                    nc.gpsimd.memset(
                        final_pad[c][:, 8:40, 40:48].bitcast(U16), 0)
                fscr = [stage.tile([128, 32, 32], F32, name=f"fscr{i}")
                        for i in range(2)]
                for cb in range(2):
                    for ch in range(2):
                        r0 = 16 * ch
                        nc.vector.tensor_tensor(
                            fscr[0][:, r0:r0 + 16, :], prec[cb][ch][:],
                            maskb9s[:, r0:r0 + 16, :], ALU.mult)
                    nc.vector.tensor_tensor(fscr[0][:], fscr[0][:],
                                            s2b[:], ALU.add)
                    nc.gpsimd.tensor_tensor(fscr[1][:], fflat[:, cb],
                                            invmaskb[:], ALU.mult)
                    nc.vector.tensor_tensor(
                        final_pad[cb][:, 8:40, 8:40],
                        fscr[0][:], fscr[1][:], ALU.add)
            ps_db_cm.__exit__(None, None, None)
            ps_acc_cm.__exit__(None, None, None)
            stage_cm.__exit__(None, None, None)
        # ---------- work pool closed ----------

        with tc.tile_pool(name="late", bufs=1) as late:
            # ---------- dilated convs (bf16) ----------
            wsb = [late.tile([128, 2304], BF16, name=f"wsb{c}") for c in range(2)]
            biasb = [late.tile([128, 1], F32, name=f"biasb{c}") for c in range(2)]
            for c in range(2):
                nc.scalar.dma_start(wsb[c][:], w_d[c])
                nc.scalar.dma_start(biasb[c][:], b_d[128 * c:128 * (c + 1)])
            out_sb = [late.tile([128, 32, 32], F32, name=f"out_sb{c}")
                      for c in range(2)]

            with tc.tile_pool(name="ps_o", bufs=3, space="PSUM") as pso_pool:
                for ct_out in range(2):
                    for ch in range(2):
                        pso = pso_pool.tile([128, 512], F32, name="pso",
                                            tag="pso")
                        for half in range(2):
                            g = 2 * ct_out + half
                            r = RATES[g]
                            i = 0
                            for c in range(2):
                                for d, (dy, dx) in enumerate(OFFS):
                                    oy = 8 + r * (dy - 1) + 16 * ch
                                    ox = 8 + r * (dx - 1)
                                    woff = 576 * g + 64 * (3 * dy + dx)
                                    nc.tensor.matmul(
                                        pso[64 * half:64 * half + 64, :],
                                        wsb[c][:, woff:woff + 64],
                                        final_pad[c][:, oy:oy + 16, ox:ox + 32],
                                        start=(i == 0), stop=(i == 17),
                                        tile_position=(0, 64 * half))
                                    i += 1
                        # split the last chunk so its DMA overlaps the relu
                        nsub = 2 if (ct_out == 1 and ch == 1) else 1
                        for s in range(nsub):
                            rs = 16 * ch + (16 // nsub) * s
                            rn_ = 16 // nsub
                            nc.scalar.activation(
                                out_sb[ct_out][:, rs:rs + rn_, :],
                                pso[:].rearrange("p (a b) -> p a b", b=32)
                                [:, rs - 16 * ch:rs - 16 * ch + rn_, :],
                                AF.Relu, bias=biasb[ct_out][:])
                            (nc.sync if (ch == 0 or s == 1) else
                             nc.scalar).dma_start(
                                out_d[128 * ct_out:128 * (ct_out + 1),
                                      rs:rs + rn_, :],
                                out_sb[ct_out][:, rs:rs + rn_, :])


def _get_nc():
    if "nc" not in _CACHE:
        _CACHE["nc"] = build_program()
    return _CACHE["nc"]


def kernel(foreground, mask, background, conv_w, conv_b):
    nc = _get_nc()
    fg = np.ascontiguousarray(foreground, dtype=np.float32).reshape(
        8, 2, 128, 32, 32).astype(ml_dtypes.bfloat16)
    bg = np.ascontiguousarray(background, dtype=np.float32).reshape(
        8, 2, 128, 32, 32).astype(ml_dtypes.bfloat16)
    maskrow = np.ascontiguousarray(mask.reshape(1, 1024), dtype=np.float32)
    # conv_w [4,64,256,3,3] -> [c, g, dy, dx, o] -> [2, 128, 2304] bf16
    wre = np.ascontiguousarray(
        conv_w.astype(np.float32).transpose(2, 0, 3, 4, 1).reshape(2, 128, 2304)
    ).astype(ml_dtypes.bfloat16)
    bias = np.ascontiguousarray(conv_b.astype(np.float32).reshape(256, 1))
    in_maps = [
        {"fg": fg[i], "bg": bg[i], "maskrow": maskrow, "wconv": wre, "bias": bias}
        for i in range(8)
    ]
    res = run_bass_kernel_spmd(nc, in_maps, list(range(8)))
    return np.stack([res.results[i]["out"] for i in range(8)], axis=0)


if __name__ == "__main__":
    build_program()
    print("build ok")


# revision 39
# speedup vs baseline: 1.5704x; 1.0617x over previous
"""Trainium2 Bass kernel for ContextualAttentionModule.

Data-parallel over batch: 8 samples -> 8 NeuronCores, one sample per core.
Per-core pipeline (C=256, H=W=32, L=1024 patches):
  scores  = <fg_patch(p), bg_patch(l)> via fp8e4 DoubleRow matmuls, hi-term
            only (the recovered/attention branch carries ~3% of output
            energy, so fp8 rounding there is far under the error budget)
  norm    = rsqrt(3x3-boxsum(colsum bg_masked^2) + 2304*eps^2), bf16 boxsum
  prop    = 3x3 window-sum of scores + eps*boxbox(colsum fg)  (DVE, bf16)
  E       = exp(prop * rn) in bf16 (no max-subtract)
  D, s2   = interleaved per-block [1,512] PE accumulations over E (lag 2)
  T8      = fp8(E * (1/D) * min(rn*2048, 256))  -- the 2^11 gain keeps the
            attn operand in fp8's normal range; undone exactly via the
            mask/(9*2048) constant tile.  1/D read directly from PSUM.
  recov   = fp8 DoubleRow tconv: t-block pairs contract in one matmul
            (lhs = transposed masked-bg banks quantized to fp8, rhs = T8)
  final   = (recov*mask/(9*2048)*2048 + eps*box(s2/D))*... + fg*(1-mask)
  out     = concat_g relu(dilated_conv_r(final) + b)  bf16 weights
"""

import numpy as np
import ml_dtypes

import concourse.bass as bass
import concourse.tile as tile
from concourse import bacc, mybir
from concourse.bass_utils import run_bass_kernel_spmd
from concourse.masks import make_identity

F32 = mybir.dt.float32
F32R = mybir.dt.float32r
BF16 = mybir.dt.bfloat16
F8 = mybir.dt.float8e4
U8 = mybir.dt.uint8
U16 = mybir.dt.uint16
AF = mybir.ActivationFunctionType
ALU = mybir.AluOpType
DR = mybir.MatmulPerfMode.DoubleRow

EPS = 1e-7
TS = 2048.0          # power-of-2 gain on the attn operand for fp8 range
RATES = (1, 2, 4, 8)
OFFS = [(dy, dx) for dy in range(3) for dx in range(3)]
# dy=1 taps first so the start=True matmul covers the full psum tile
TAPS = [(1, 0), (1, 1), (1, 2), (0, 0), (0, 1), (0, 2), (2, 0), (2, 1), (2, 2)]

_CACHE = {}


def _ring_zero(nc, buf, n=34, eng=None):
    """Zero only the 1-wide border ring of a [P, n, n] padded buffer."""
    eng = eng or nc.vector
    bc = U16 if buf.dtype == BF16 else F32
    eng.memset(buf[:, 0:n:n - 1, :].bitcast(bc), 0)
    eng.memset(buf[:, 1:n - 1, 0:n:n - 1].bitcast(bc), 0)


def _ring_zero_pair(nc, buf, eng=None):
    """Ring-zero both j-halves of a [P, 2, 34, 34] paired buffer."""
    eng = eng or nc.vector
    bc = U16 if buf.dtype == BF16 else U8
    eng.memset(buf[:, :, 0:34:33, :].bitcast(bc), 0)
    eng.memset(buf[:, :, 1:33, 0:34:33].bitcast(bc), 0)


def _boxsum(nc, scr, src_pad, dst_flat, eng=None):
    """3x3 SAME window sum: [1,34,34] ring-zero padded -> [1,32,32] flat."""
    eng = eng or nc.vector
    eng.tensor_tensor(scr[:, 1:33, 1:33], src_pad[:, 1:33, 0:32],
                      src_pad[:, 1:33, 1:33], ALU.add)
    eng.tensor_tensor(scr[:, 1:33, 1:33], scr[:, 1:33, 1:33],
                      src_pad[:, 1:33, 2:34], ALU.add)
    eng.tensor_tensor(dst_flat[:], scr[:, 0:32, 1:33],
                      scr[:, 1:33, 1:33], ALU.add)
    eng.tensor_tensor(dst_flat[:], dst_flat[:], scr[:, 2:34, 1:33], ALU.add)


def build_program():
    nc = bacc.Bacc()
    fg_d = nc.declare_dram_parameter("fg", [2, 128, 32, 32], BF16, isOutput=False)
    fg8_d = nc.declare_dram_parameter("fg8", [3, 128, 2, 32, 32], F8,
                                      isOutput=False)
    bg_d = nc.declare_dram_parameter("bg", [2, 128, 32, 32], BF16, isOutput=False)
    mask_d = nc.declare_dram_parameter("maskrow", [1, 1024], F32, isOutput=False)
    w_d = nc.declare_dram_parameter("wconv", [2, 128, 2304], BF16, isOutput=False)
    b_d = nc.declare_dram_parameter("bias", [256, 1], F32, isOutput=False)
    out_d = nc.declare_dram_parameter("out", [256, 32, 32], F32, isOutput=True)

    with tile.TileContext(nc) as tc:
        _emit(nc, tc, fg_d, fg8_d, bg_d, mask_d, w_d, b_d, out_d)
    nc.compile()
    return nc


def _emit(nc, tc, fg_d, fg8_d, bg_d, mask_d, w_d, b_d, out_d):
    with tc.tile_pool(name="main", bufs=1) as main:
        # ---------------- long-lived tiles ----------------
        fflat = main.tile([128, 2, 32, 32], BF16, name="fflat")
        invmaskb = main.tile([128, 32, 32], F32, name="invmaskb")
        maskb9 = main.tile([128, 32, 32], F32, name="maskb9")
        maskb9s = main.tile([128, 32, 32], F32, name="maskb9s")
        ones_col = main.tile([128, 1], F32R, name="ones_col")
        idR = main.tile([128, 128], BF16, name="idR")
        rncol = main.tile([128, 8], F32, name="rncol")
        rnccol = main.tile([128, 8], F32, name="rnccol")
        ercol = main.tile([128, 8], BF16, name="ercol")
        onesB = main.tile([128, 1], BF16, name="onesB")
        Gb = main.tile([128, 32, 32], BF16, name="Gb")
        msrow = main.tile([1, 1024], F32, name="msrow")
        Ap = [main.tile([128, 2, 34, 34], BF16, name=f"Ap{q}") for q in range(4)]
        A = [Ap[t // 2][:, t % 2] for t in range(8)]
        T8p = [main.tile([128, 2, 34, 34], F8, name=f"T8p{q}") for q in range(4)]
        W = [main.tile([128, 34, 34], BF16, name=f"W{i}") for i in range(2)]

        with tc.tile_pool(name="work", bufs=1) as work:
            # masked-bg x-shift variants: tile v holds value x[w + v - 1]
            # (0 at edges); rows padded (34) so dy comes from row offsets.
            bgb = [work.tile([128, 2, 34, 32], BF16, name=f"bgb{v}")
                   for v in range(3)]
            bg8h = [work.tile([128, 2, 34, 32], F8, name=f"bg8h{v}")
                    for v in range(3)]
            fg8h = [work.tile([128, 2, 32, 32], F8, name=f"fg8h{v}")
                    for v in range(3)]
            bgT8 = [work.tile([128, 2, 2304], F8, name=f"bgT8{q}")
                    for q in range(4)]
            rdrow = work.tile([1, 1024], F32, name="rdrow")
            s2row = work.tile([1, 1024], F32, name="s2row")

            stage_cm = tc.tile_pool(name="stage", bufs=1)
            stage = stage_cm.__enter__()
            ps_acc_cm = tc.tile_pool(name="ps_acc", bufs=1, space="PSUM")
            ps_acc = ps_acc_cm.__enter__()
            ps_sc_cm = tc.tile_pool(name="ps_sc", bufs=2, space="PSUM")
            ps_sc = ps_sc_cm.__enter__()
            ps_tra_cm = tc.tile_pool(name="ps_tra", bufs=3, space="PSUM")
            ps_tra = ps_tra_cm.__enter__()

            bq = stage.tile([128, 2, 32, 32], BF16, name="bq")
            q2 = stage.tile([128, 2, 32, 32], F32R, name="q2")
            onesf0 = stage.tile([128, 1], F32, name="onesf0")

            # ---------- phase 0: loads + edge zeros + quantize ----------
            nc.sync.dma_start(msrow[:], mask_d[:])
            nc.gpsimd.dma_start(bq[:, 0], bg_d[0])
            nc.sync.dma_start(bq[:, 1], bg_d[1])
            nc.scalar.dma_start(fg8h[1][:], fg8_d[1])
            nc.sync.dma_start(fg8h[0][:], fg8_d[0])
            nc.scalar.dma_start(fg8h[2][:], fg8_d[2])
            nc.sync.dma_start(fflat[:, 1], fg_d[1])
            nc.scalar.dma_start(fflat[:, 0], fg_d[0])

            nc.gpsimd.memset(onesf0[:], 1.0)
            # preload the ln+exp activation table once; every activation in
            # this kernel (Copy/Square/Ln/Exp/Relu) lives in it.
            nc.scalar.activation(onesf0[0:1], onesf0[0:1], AF.Ln)
            nc.scalar.activation(onesf0[0:1], onesf0[0:1], AF.Exp)
            nc.gpsimd.memset(onesf0[:], 1.0)

            # load-independent zero-fills while DMAs fly
            onesrowR = stage.tile([1, 128], F32R, name="onesrowR")
            nc.vector.memset(onesrowR[:].bitcast(F32), 1.0)
            idf = stage.tile([128, 128], F32, name="idf")
            make_identity(nc, idf[:])
            nc.gpsimd.tensor_copy(idR[:], idf[:])
            nc.vector.tensor_copy(ones_col[:], onesf0[:])
            nc.gpsimd.tensor_copy(onesB[:], onesf0[:])
            for q in range(4):
                _ring_zero_pair(nc, Ap[q], eng=(nc.vector if q % 2 else nc.gpsimd))
                _ring_zero_pair(nc, T8p[q], eng=(nc.gpsimd if q % 2 else nc.vector))
            for w in W:
                _ring_zero(nc, w, eng=nc.gpsimd)
            for lst, edge in ((bgb, U16), (bg8h, U8)):
                nc.vector.memset(lst[0][:, :, 0:34:33, :].bitcast(edge), 0)
                nc.gpsimd.memset(lst[1][:, :, 0:34:33, :].bitcast(edge), 0)
                nc.vector.memset(lst[2][:, :, 0:34:33, :].bitcast(edge), 0)
                nc.vector.memset(lst[0][:, :, 1:33, 0:1].bitcast(edge), 0)
                nc.gpsimd.memset(lst[2][:, :, 1:33, 31:32].bitcast(edge), 0)

            # mask broadcast via PE ones-outer-product
            msrowR = stage.tile([1, 1024], F32R, name="msrowR")
            nc.vector.tensor_copy(msrowR[:], msrow[:])
            invmaskB = stage.tile([128, 32, 32], BF16, name="invmaskB")
            for h in range(2):
                psb = ps_sc.tile([128, 16, 32], F32, name="psb", tag="psc")
                nc.tensor.matmul(
                    psb[:].rearrange("p a b -> p (a b)"), onesrowR[:],
                    msrowR[:, 512 * h:512 * (h + 1)], start=True, stop=True)
                nc.vector.tensor_scalar(
                    out=invmaskB[:, 16 * h:16 * (h + 1), :], in0=psb[:],
                    scalar1=-1.0, scalar2=1.0, op0=ALU.mult, op1=ALU.add)

            # q2 = bg^2 (unmasked; per-pixel mask applied on the ssq row)
            nc.scalar.activation(q2[:, 0], bq[:, 0], AF.Square)
            nc.scalar.activation(q2[:, 1], bq[:, 1], AF.Square)

            # masked bg x-variants, bf16 (transpose source) -- STT from raw
            # loads; fp8 copies for the scores lhs banks.
            SHIFT = {0: (1, 32, 0, 31), 1: (0, 32, 0, 32), 2: (0, 31, 1, 32)}
            for v in range(3):
                c0, c1, s0, s1 = SHIFT[v]
                for cb in range(2):
                    nc.vector.tensor_tensor(
                        bgb[v][:, cb, 1:33, c0:c1], bq[:, cb, :, s0:s1],
                        invmaskB[:, :, s0:s1], ALU.mult)
            for v in range(3):
                for cb in range(2):
                    if (v, cb) in ((0, 0), (2, 1)):
                        nc.vector.tensor_copy(bg8h[v][:, cb], bgb[v][:, cb])
                    else:
                        nc.scalar.copy(bg8h[v][:, cb], bgb[v][:, cb])

            # ---------- phase 1: patch norms + Gb (emitted before scores
            # so its PE matmuls run while the quantize chain finishes) ----
            rpad = stage.tile([1, 34, 34], BF16, name="rpad")
            rscr = stage.tile([1, 34, 34], BF16, name="rscr")
            _ring_zero(nc, rpad, eng=nc.gpsimd)
            _ring_zero(nc, rscr, eng=nc.gpsimd)
            invrow = stage.tile([1, 1024], F32, name="invrow")
            nc.vector.tensor_scalar(out=invrow[:], in0=msrow[:],
                                    scalar1=-1.0, scalar2=1.0,
                                    op0=ALU.mult, op1=ALU.add)
            sqrow = stage.tile([1, 1024], F32, name="sqrow")
            for ch in range(2):
                pr = ps_sc.tile([1, 512], F32, name="pr", tag="psc")
                for cb in range(2):
                    nc.tensor.matmul(
                        pr[:], ones_col[:],
                        q2[:, cb, 16 * ch:16 * ch + 16, :],
                        start=(cb == 0), stop=(cb == 1))
                # mask per-pixel on the row, then stash into padded grid
                nc.vector.tensor_tensor(
                    sqrow[:, 512 * ch:512 * (ch + 1)], pr[:],
                    invrow[:, 512 * ch:512 * (ch + 1)], ALU.mult)
                nc.scalar.copy(
                    rpad[:, 1 + 16 * ch:17 + 16 * ch, 1:33],
                    sqrow[:, 512 * ch:512 * (ch + 1)].rearrange(
                        "o (a b) -> o a b", b=32))
            ssqrow = stage.tile([1, 32, 32], BF16, name="ssqrow")
            _boxsum(nc, rscr, rpad, ssqrow)
            urow = stage.tile([1, 1024], F32, name="urow")
            nc.vector.tensor_scalar_add(
                urow[:], ssqrow[:].rearrange("o a b -> o (a b)"),
                2304.0 * EPS * EPS)
            # rn = 1/sqrt(u) = exp(-0.5 * ln u): stays in the ln+exp table
            nc.scalar.activation(urow[:], urow[:], AF.Ln)
            nc.scalar.activation(urow[:], urow[:], AF.Exp, scale=-0.5)
            # rncol[p, t] = rnrow[128t + p]: broadcast + diag extract
            Bb = stage.tile([128, 1024], F32, name="Bb")
            nc.gpsimd.partition_broadcast(Bb[:], urow[:])
            junk = stage.tile([128, 128], F32, name="junk")
            for t in range(8):
                nc.vector.scalar_tensor_tensor(
                    out=junk[:], in0=Bb[:, 128 * t:128 * (t + 1)],
                    scalar=1.0, in1=idf[:], op0=ALU.mult, op1=ALU.mult,
                    accum_out=rncol[:, t:t + 1])
            nc.vector.tensor_scalar(out=rnccol[:], in0=rncol[:], scalar1=TS,
                                    scalar2=256.0, op0=ALU.mult, op1=ALU.min)
            nc.vector.tensor_scalar_mul(ercol[:], rncol[:], EPS)

            # Gs = eps * 3x3-box of channel-sum(fg), broadcast.  Added into
            # A alongside the psc copy; the padded-ring boxsum over A then
            # supplies the outer box of the reference's eps*boxbox term.
            for ch in range(2):
                prg = ps_sc.tile([1, 512], F32, name="prg", tag="psc")
                for cb in range(2):
                    nc.tensor.matmul(
                        prg[:], onesB[:],
                        fflat[:, cb, 16 * ch:16 * ch + 16, :],
                        start=(cb == 0), stop=(cb == 1))
                nc.scalar.copy(rpad[:, 1 + 16 * ch:17 + 16 * ch, 1:33], prg[:])
            _boxsum(nc, rscr, rpad, ssqrow)
            ebrowB = stage.tile([1, 1024], BF16, name="ebrowB")
            nc.vector.tensor_scalar_mul(
                ebrowB[:], ssqrow[:].rearrange("o a b -> o (a b)"), EPS)
            nc.gpsimd.partition_broadcast(
                Gb.rearrange("p a b -> p (a b)"), ebrowB[:])

            # mask constants (off critical path)
            nc.gpsimd.tensor_copy(invmaskb[:], invmaskB[:])
            nc.gpsimd.tensor_scalar(
                out=maskb9[:], in0=invmaskB[:], scalar1=-1.0 / 9.0,
                scalar2=1.0 / 9.0, op0=ALU.mult, op1=ALU.add)
            nc.gpsimd.tensor_scalar(
                out=maskb9s[:], in0=invmaskB[:], scalar1=-1.0 / (9.0 * TS),
                scalar2=1.0 / (9.0 * TS), op0=ALU.mult, op1=ALU.add)

            # ---------- scores loop (fp8 DR, hi only) ----------
            # row-accumulators pair up in PSUM banks via partition offsets
            accD = ps_acc.tile([128, 512], F32, name="accD")
            accS = ps_acc.tile([128, 512], F32, name="accS")
            dAcc = [accD[64 * c:64 * c + 1, :] for c in range(2)]
            s2Acc = [accS[64 * c:64 * c + 1, :] for c in range(2)]

            def emit_scores(t, ch):
                psc = ps_sc.tile([128, 16, 32], F32, name="psc", tag="psc")
                for i, (dy, dx) in enumerate(TAPS):
                    fr = 16 * ch + dy - 1
                    r0, r1 = max(0, -fr), min(16, 32 - fr)
                    nc.tensor.matmul(
                        psc[:, r0:r1, :],
                        bg8h[dx][:, :, 4 * t + dy:4 * t + dy + 4, :],
                        fg8h[dx][:, :, fr + r0:fr + r1, :],
                        start=(i == 0), stop=(i == 8), perf_mode=DR)
                half = A[t][:, 1 + 16 * ch:17 + 16 * ch, 1:33]
                nc.vector.tensor_tensor(
                    half, psc[:], Gb[:, 16 * ch:16 * ch + 16, :], ALU.add)

            hp_cm = tc.tile_pool(name="hp", bufs=2)
            hp = hp_cm.__enter__()

            def emit_boxsum_exp(t):
                w = W[t % 2]
                nc.vector.tensor_tensor(
                    w[:, 1:33, 1:33], A[t][:, 1:33, 0:32],
                    A[t][:, 1:33, 1:33], ALU.add)
                nc.vector.tensor_tensor(
                    w[:, 1:33, 1:33], w[:, 1:33, 1:33],
                    A[t][:, 1:33, 2:34], ALU.add)
                H = hp.tile([128, 32, 32], BF16, name="H", tag="H")
                nc.vector.tensor_tensor(
                    H[:], w[:, 0:32, 1:33], w[:, 1:33, 1:33], ALU.add)
                nc.vector.tensor_tensor(H[:], H[:], w[:, 2:34, 1:33], ALU.add)
                nc.scalar.activation(A[t][:, 1:33, 1:33], H[:],
                                     AF.Exp, scale=rncol[:, t:t + 1])

            def emit_d_s2(t):
                for ch in range(2):
                    av = A[t][:, 1 + 16 * ch:17 + 16 * ch, 1:33]
                    nc.tensor.matmul(dAcc[ch][:], onesB[:], av,
                                     start=(t == 0), stop=(t == 7))
                    nc.tensor.matmul(s2Acc[ch][:], ercol[:, t:t + 1],
                                     av, start=(t == 0), stop=(t == 7))

            def build_bgT8(q):
                # transpose (j, cb, d) 128x128 blocks of shifted masked bg
                # into bf16 psum, 4 at a time, then fp8-quantize to SBUF.
                blocks = [(j, cb, d) for j in range(2) for cb in range(2)
                          for d in range(9)]
                for g in range(9):
                    chunk = blocks[4 * g:4 * g + 4]
                    ptr = ps_tra.tile([128, 512], BF16, name="ptr", tag="ptr")
                    for bi, (j, cb, d) in enumerate(chunk):
                        dy, dx = OFFS[d]
                        t = 2 * q + j
                        nc.tensor.transpose(
                            ptr[:, 128 * bi:128 * (bi + 1)],
                            bgb[dx][:, cb, 4 * t + dy:4 * t + dy + 4, :]
                            .rearrange("p a b -> p (a b)"),
                            idR[:])
                    dst = bgT8[q].rearrange("p j c -> p (j c)")[
                        :, 512 * g:512 * g + 512]
                    if g % 3 < 2:
                        nc.scalar.copy(dst, ptr[:])
                    else:
                        nc.vector.tensor_copy(dst, ptr[:])

            for t in range(8):
                emit_scores(t, 0)
                if t >= 3:
                    emit_d_s2(t - 3)
                emit_scores(t, 1)
                if t < 4:
                    build_bgT8(t)
                emit_boxsum_exp(t)
            emit_d_s2(5)
            emit_d_s2(6)
            emit_d_s2(7)
            hp_cm.__exit__(None, None, None)
            ps_tra_cm.__exit__(None, None, None)
            ps_sc_cm.__exit__(None, None, None)

            # ---------- softmax denom + T8 + tconv ----------
            ps_db_cm = tc.tile_pool(name="ps_db", bufs=1, space="PSUM")
            ps_db = ps_db_cm.__enter__()
            psd = [ps_db.tile([128, 512], F32, name=f"psd{c}") for c in range(2)]
            rdrowR = stage.tile([1, 1024], F32R, name="rdrowR")
            for ch in range(2):
                nc.vector.reciprocal(
                    rdrow[:, 512 * ch:512 * (ch + 1)], dAcc[ch][:])
                nc.vector.tensor_copy(rdrowR[:, 512 * ch:512 * (ch + 1)],
                                      rdrow[:, 512 * ch:512 * (ch + 1)])
                nc.tensor.matmul(psd[ch][:], onesrowR[:],
                                 rdrowR[:, 512 * ch:512 * (ch + 1)],
                                 start=True, stop=True)
                nc.vector.tensor_tensor(
                    s2row[:, 512 * ch:512 * (ch + 1)], s2Acc[ch][:],
                    rdrow[:, 512 * ch:512 * (ch + 1)], ALU.mult)

            Db = stage.tile([128, 32, 32], BF16, name="Db")
            for ch in range(2):
                nc.scalar.copy(
                    Db[:, 16 * ch:16 * ch + 16, :],
                    psd[ch][:].rearrange("p (a b) -> p a b", b=32))

            def emit_T8(t):
                q, j = t // 2, t % 2
                nc.vector.scalar_tensor_tensor(
                    out=T8p[q][:, j, 1:33, 1:33],
                    in0=A[t][:, 1:33, 1:33],
                    scalar=rnccol[:, t:t + 1],
                    in1=Db[:],
                    op0=ALU.mult, op1=ALU.mult)

            # s2 path early: runs on Pool while T8/tconv proceed
            s2pad = W[0][0:1]
            s2scr = W[1][0:1]
            _ring_zero(nc, s2pad, eng=nc.gpsimd)
            _ring_zero(nc, s2scr, eng=nc.gpsimd)
            nc.gpsimd.tensor_copy(
                s2pad[:, 1:33, 1:33],
                s2row[:].rearrange("o (a b) -> o a b", b=32))
            boxs2 = stage.tile([1, 32, 32], BF16, name="boxs2")
            _boxsum(nc, s2scr, s2pad, boxs2, eng=nc.gpsimd)
            nc.gpsimd.tensor_scalar_mul(
                s2row[:], boxs2[:].rearrange("o a b -> o (a b)"), EPS)
            s2b = stage.tile([128, 32, 32], F32, name="s2b")
            nc.gpsimd.partition_broadcast(
                s2b.rearrange("p a b -> p (a b)"), s2row[:])
            nc.gpsimd.tensor_tensor(s2b[:], s2b[:], maskb9[:], ALU.mult)

            with tc.tile_pool(name="ps_rec", bufs=1, space="PSUM") as psrec:
                prec = [[psrec.tile([128, 512], F32, name=f"prec{c}_{ch}")
                         for ch in range(2)] for c in range(2)]

                def emit_tconv(q):
                    for cb in range(2):
                        for ch in range(2):
                            for d, (dy, dx) in enumerate(OFFS):
                                z0 = 16 * ch + 2 - dy
                                x0 = 2 - dx
                                nc.tensor.matmul(
                                    prec[cb][ch][:],
                                    bgT8[q][:, :, 128 * (9 * cb + d):
                                            128 * (9 * cb + d + 1)],
                                    T8p[q][:, :, z0:z0 + 16, x0:x0 + 32],
                                    start=(q == 0 and d == 0),
                                    stop=(q == 3 and d == 8), perf_mode=DR)

                for t in range(8):
                    emit_T8(t)
                for q in range(4):
                    emit_tconv(q)

                # final = prec*mask/(9*TS) + eps*boxs2*mask/9 + fg*(1-mask)
                final_pad = [main.tile([128, 48, 48], BF16,
                                       name=f"final_pad{c}") for c in range(2)]
                for c in range(2):
                    nc.vector.memset(final_pad[c][:, 0:8, :].bitcast(U16), 0)
                    nc.vector.memset(final_pad[c][:, 40:48, :].bitcast(U16), 0)
                    nc.gpsimd.memset(
                        final_pad[c][:, 8:40, 0:8].bitcast(U16), 0)
                    nc.gpsimd.memset(
                        final_pad[c][:, 8:40, 40:48].bitcast(U16), 0)
                fscr = [stage.tile([128, 32, 32], F32, name=f"fscr{i}")
                        for i in range(2)]
                for cb in range(2):
                    for ch in range(2):
                        r0 = 16 * ch
                        nc.vector.tensor_tensor(
                            fscr[0][:, r0:r0 + 16, :], prec[cb][ch][:],
                            maskb9s[:, r0:r0 + 16, :], ALU.mult)
                    nc.vector.tensor_tensor(fscr[0][:], fscr[0][:],
                                            s2b[:], ALU.add)
                    nc.gpsimd.tensor_tensor(fscr[1][:], fflat[:, cb],
                                            invmaskb[:], ALU.mult)
                    nc.vector.tensor_tensor(
                        final_pad[cb][:, 8:40, 8:40],
                        fscr[0][:], fscr[1][:], ALU.add)
            ps_db_cm.__exit__(None, None, None)
            ps_acc_cm.__exit__(None, None, None)
            stage_cm.__exit__(None, None, None)
        # ---------- work pool closed ----------

        with tc.tile_pool(name="late", bufs=1) as late:
            # ---------- dilated convs (bf16) ----------
            wsb = [late.tile([128, 2304], BF16, name=f"wsb{c}") for c in range(2)]
            biasb = [late.tile([128, 1], F32, name=f"biasb{c}") for c in range(2)]
            for c in range(2):
                nc.scalar.dma_start(wsb[c][:], w_d[c])
                nc.scalar.dma_start(biasb[c][:], b_d[128 * c:128 * (c + 1)])
            out_sb = [late.tile([128, 32, 32], F32, name=f"out_sb{c}")
                      for c in range(2)]

            with tc.tile_pool(name="ps_o", bufs=3, space="PSUM") as pso_pool:
                for ct_out in range(2):
                    for ch in range(2):
                        pso = pso_pool.tile([128, 512], F32, name="pso",
                                            tag="pso")
                        for half in range(2):
                            g = 2 * ct_out + half
                            r = RATES[g]
                            i = 0
                            for c in range(2):
                                for d, (dy, dx) in enumerate(OFFS):
                                    oy = 8 + r * (dy - 1) + 16 * ch
                                    ox = 8 + r * (dx - 1)
                                    woff = 576 * g + 64 * (3 * dy + dx)
                                    nc.tensor.matmul(
                                        pso[64 * half:64 * half + 64, :],
                                        wsb[c][:, woff:woff + 64],
                                        final_pad[c][:, oy:oy + 16, ox:ox + 32],
                                        start=(i == 0), stop=(i == 17),
                                        tile_position=(0, 64 * half))
                                    i += 1
                        # split the last chunk so its DMA overlaps the relu
                        nsub = 2 if (ct_out == 1 and ch == 1) else 1
                        for s in range(nsub):
                            rs = 16 * ch + (16 // nsub) * s
                            rn_ = 16 // nsub
                            nc.scalar.activation(
                                out_sb[ct_out][:, rs:rs + rn_, :],
                                pso[:].rearrange("p (a b) -> p a b", b=32)
                                [:, rs - 16 * ch:rs - 16 * ch + rn_, :],
                                AF.Relu, bias=biasb[ct_out][:])
                            nc.sync.dma_start(
                                out_d[128 * ct_out:128 * (ct_out + 1),
                                      rs:rs + rn_, :],
                                out_sb[ct_out][:, rs:rs + rn_, :])


def _get_nc():
    if "nc" not in _CACHE:
        _CACHE["nc"] = build_program()
    return _CACHE["nc"]


def kernel(foreground, mask, background, conv_w, conv_b):
    nc = _get_nc()
    fg = np.ascontiguousarray(foreground, dtype=np.float32).reshape(
        8, 2, 128, 32, 32).astype(ml_dtypes.bfloat16)
    bg = np.ascontiguousarray(background, dtype=np.float32).reshape(
        8, 2, 128, 32, 32).astype(ml_dtypes.bfloat16)
    # pre-shifted fp8 fg variants: fg8[v][..., x] = fg[..., x + v - 1]
    f8c = fg.astype(ml_dtypes.float8_e4m3)          # [8, 2, 128, 32, 32]
    z = np.zeros_like(f8c[..., :1])
    fg8 = np.stack([
        np.concatenate([z, f8c[..., :-1]], axis=-1),
        f8c,
        np.concatenate([f8c[..., 1:], z], axis=-1),
    ], axis=1).transpose(0, 1, 3, 2, 4, 5)          # [8, 3, 128, 2, 32, 32]
    fg8 = np.ascontiguousarray(fg8)
    maskrow = np.ascontiguousarray(mask.reshape(1, 1024), dtype=np.float32)
    # conv_w [4,64,256,3,3] -> [c, g, dy, dx, o] -> [2, 128, 2304] bf16
    wre = np.ascontiguousarray(
        conv_w.astype(np.float32).transpose(2, 0, 3, 4, 1).reshape(2, 128, 2304)
    ).astype(ml_dtypes.bfloat16)
    bias = np.ascontiguousarray(conv_b.astype(np.float32).reshape(256, 1))
    in_maps = [
        {"fg": fg[i], "fg8": fg8[i], "bg": bg[i], "maskrow": maskrow,
         "wconv": wre, "bias": bias}
        for i in range(8)
    ]
    res = run_bass_kernel_spmd(nc, in_maps, list(range(8)))
    return np.stack([res.results[i]["out"] for i in range(8)], axis=0)


if __name__ == "__main__":
    build_program()
    print("build ok")
